# revision 55
# baseline (speedup 1.0000x reference)
"""GNN message-passing layer (EquivariantMPLayer) on 8 Trainium2 NeuronCores.

Sharding: edges are sharded by destination-node range (dst // (N/8)) so each
core aggregates its own node range locally -- no collectives needed. Per core,
edges are sorted by dst and grouped into 128-node windows; each window's edge
list is padded to 128-edge blocks. Per-window block counts are equalized
across cores (max over cores) so a single SPMD program serves all 8 cores.

The host pre-gathers x[src] and x[dst] for every edge slot into a single
feature-major stream xcatT [128, epad] (rows 0:64 = src feats, 64:128 = dst
feats, columns in device consumption order), and pre-builds the per-block
scatter one-hots as a second stream ohsT [128, epad] whose nonzero VALUES are
1/cnt(dst) -- so the scatter matmul directly produces the mean aggregate.

Algebraic folds: the L2 weight is W2U = mw2 @ uw1_agg, so the scatter PSUM
accumulates uw1_agg^T @ (agg/cnt) -- the update-MLP's aggregation term --
directly. When a window closes, one extra matmul adds uw1_x^T @ x (with an
augmented constant row supplying has*uw1_agg^T@mb2 exactly), and a single
ACT Silu produces the u1 hidden vector for those 128 nodes. The second
update layer + LayerNorm run as a short pipelined end phase.

The device pipeline is software-pipelined so the tensor engine never waits:
at iteration k it runs L1(k), L2(k-2) and scatter(k-6), while ACT runs
silu(k) and the per-window u1 activation, and DVE only copies msg out of
PSUM.
"""

import numpy as np

N = 50000
E = 800000
DIN = 64
DOUT = 64
NB = 16
MAX_RADIUS = 10.0
NCORES = 8
P = 128

_prog_cache = {}


# ---------------------------------------------------------------------------
# Host-side structure / metadata
# ---------------------------------------------------------------------------

def _build_host_data(x, edge_index, edge_len, mw1, mb1, mw2, mb2,
                     uw1, ub1, uw2, ub2, ln_g, ln_b,
                     n=N, ncores=NCORES):
    import ml_dtypes
    bf16 = ml_dtypes.bfloat16

    nloc = n // ncores
    nw = (nloc + P - 1) // P
    npad = nw * P

    src = np.asarray(edge_index[0], dtype=np.int64)
    dst = np.asarray(edge_index[1], dtype=np.int64)
    x = np.asarray(x, dtype=np.float32)
    el = np.asarray(edge_len, dtype=np.float32)[:, 0]

    centers = np.linspace(0.0, MAX_RADIUS, NB, dtype=np.float64)
    width = (centers[1] - centers[0]) * 0.5
    rbf_all = np.exp(-((el[:, None].astype(np.float64) - centers) ** 2)
                     / (2.0 * width ** 2)).astype(np.float32)  # [E, 16]

    core_of = dst // nloc
    per_core = []
    cnt_w = np.zeros((ncores, nw), dtype=np.int64)
    for c in range(ncores):
        eids = np.nonzero(core_of == c)[0]
        dloc = (dst[eids] - c * nloc).astype(np.int64)
        order = np.argsort(dloc, kind="stable")
        eids = eids[order]
        dloc = dloc[order]
        w_of = dloc // P
        cnt_w[c] = np.bincount(w_of, minlength=nw)
        per_core.append((eids, dloc, w_of))

    # per-window block counts, equalized across cores; total padded to x16
    # (16 blocks = one 4-group DMA chunk)
    bws = np.maximum(1, (cnt_w.max(axis=0) + P - 1) // P)  # [nw]
    bws[-1] += (-int(bws.sum())) % 16
    btot = int(bws.sum())
    epad = btot * P

    block_window = np.repeat(np.arange(nw), bws)
    boff = np.concatenate([[0], np.cumsum(bws)[:-1]])  # first block of window

    uw1f = np.asarray(uw1, np.float32)
    w2u = (np.asarray(mw2, np.float32) @ uw1f[DIN:]).astype(bf16)  # [128, 64]
    mb2u = np.asarray(mb2, np.float32) @ uw1f[DIN:]                # [64]

    in_maps = []
    for c in range(ncores):
        eids, dloc, w_of = per_core[c]
        # slot index for each edge: window base + position within window
        win_start = np.concatenate([[0], np.cumsum(cnt_w[c])[:-1]])
        pos_in_w = np.arange(len(eids)) - win_start[w_of]
        slot = boff[w_of] * P + pos_in_w  # [e_c]

        f8 = ml_dtypes.float8_e4m3

        xcat = np.zeros((epad, 2 * DIN), dtype=np.float32)
        xcat[slot, :DIN] = x[src[eids]]
        xcat[slot, DIN:] = x[dst[eids]]
        xcat8 = np.ascontiguousarray(xcat.T).astype(f8)

        rbf = np.zeros((epad, NB), dtype=np.float32)
        rbf[slot] = rbf_all[eids]
        rbf8 = np.ascontiguousarray(rbf.T).astype(f8)

        cnt_n = np.zeros(npad, dtype=np.float32)
        cnt_n[:nloc] = np.bincount(dloc, minlength=nloc).astype(np.float32)
        inv = 1.0 / np.maximum(cnt_n, 1.0)
        has = (cnt_n > 0).astype(np.float32)
        fmul = np.broadcast_to(inv[None, :], (DOUT, npad)).copy()

        # one-hot stream (fp8, exact 1.0): ohsT[lane, g*P + n] = 1 iff edge
        # slot g*P+lane scatters into window-relative node n
        oh8 = np.zeros((epad, P), dtype=np.uint8)
        oh8[slot, (dloc - w_of * P)] = 0x38  # e4m3 1.0
        ohs8 = np.ascontiguousarray(
            oh8.reshape(btot, P, P).transpose(1, 0, 2).reshape(P, epad)
        ).view(f8)

        # host-precomputed x-side of u1 pre-activation (+ mb2/ub1 folds)
        xt_loc = np.zeros((DIN, npad), dtype=np.float32)
        xt_loc[:, :nloc] = x[c * nloc:(c + 1) * nloc].T
        u1x = (uw1f[:DIN].T @ xt_loc
               + np.outer(mb2u, has)
               + np.asarray(ub1, np.float32)[:, None])

        m = {
            "xcatT": xcat8,
            "rbfT": rbf8,
            "ohsT": ohs8,
            "fmul": fmul,
            "u1x": u1x,
            "mw1_sd": np.asarray(mw1, np.float32)[:2 * DIN].astype(f8),
            "mw1_r": np.asarray(mw1, np.float32)[2 * DIN:].astype(f8),
            "mb1": np.asarray(mb1, np.float32).reshape(2 * DOUT, 1).copy(),
            "w2u": w2u,
            "uw2": np.asarray(uw2, np.float32).astype(bf16),
            "ub2": np.asarray(ub2, np.float32).reshape(DOUT, 1).copy(),
            "lng": np.broadcast_to(np.asarray(ln_g, np.float32)[None, :],
                                   (P, DOUT)).copy(),
            "lnb": np.broadcast_to(np.asarray(ln_b, np.float32)[None, :],
                                   (P, DOUT)).copy(),
            "identf": np.eye(P, dtype=np.float32),
        }
        in_maps.append(m)

    struct = dict(n=n, nloc=nloc, nw=nw, npad=npad, btot=btot, epad=epad,
                  bws=tuple(int(v) for v in bws),
                  block_window=tuple(int(v) for v in block_window))
    return struct, in_maps


# ---------------------------------------------------------------------------
# Device program
# ---------------------------------------------------------------------------

def _build_program(struct):
    import concourse.bass as bass
    import concourse.mybir as mybir
    import concourse.tile as tile
    from concourse import bacc

    f32 = mybir.dt.float32
    bf = mybir.dt.bfloat16
    f8 = mybir.dt.float8e4
    n, nloc, nw, npad = (struct["n"], struct["nloc"], struct["nw"],
                         struct["npad"])
    btot, epad = struct["btot"], struct["epad"]
    block_window = struct["block_window"]
    ngrp = btot // 4

    # first/last block of each window
    wfirst = {}
    wlast = {}
    for g, w in enumerate(block_window):
        wfirst.setdefault(w, g)
        wlast[w] = g

    nc = bacc.Bacc("TRN2", target_bir_lowering=False, debug=False,
                   enable_asserts=False, num_devices=NCORES)

    xcatT_d = nc.dram_tensor("xcatT", [P, epad], f8, kind="ExternalInput")
    rbfT_d = nc.dram_tensor("rbfT", [NB, epad], f8, kind="ExternalInput")
    ohsT_d = nc.dram_tensor("ohsT", [P, epad], f8, kind="ExternalInput")
    fmul_d = nc.dram_tensor("fmul", [DOUT, npad], f32, kind="ExternalInput")
    u1x_d = nc.dram_tensor("u1x", [DOUT, npad], f32, kind="ExternalInput")
    mw1_sd_d = nc.dram_tensor("mw1_sd", [2 * DIN, 2 * DOUT], f8,
                              kind="ExternalInput")
    mw1_r_d = nc.dram_tensor("mw1_r", [NB, 2 * DOUT], f8,
                             kind="ExternalInput")
    mb1_d = nc.dram_tensor("mb1", [2 * DOUT, 1], f32, kind="ExternalInput")
    w2u_d = nc.dram_tensor("w2u", [2 * DOUT, DOUT], bf, kind="ExternalInput")
    uw2_d = nc.dram_tensor("uw2", [DOUT, DOUT], bf, kind="ExternalInput")
    ub2_d = nc.dram_tensor("ub2", [DOUT, 1], f32, kind="ExternalInput")
    lng_d = nc.dram_tensor("lng", [P, DOUT], f32, kind="ExternalInput")
    lnb_d = nc.dram_tensor("lnb", [P, DOUT], f32, kind="ExternalInput")
    identf_d = nc.dram_tensor("identf", [P, P], f32, kind="ExternalInput")
    out_d = nc.dram_tensor("out", [npad, DOUT], f32, kind="ExternalOutput")

    AX = mybir.AxisListType
    OP = mybir.AluOpType
    ACT = mybir.ActivationFunctionType

    with tile.TileContext(nc) as tc:
        with (
            tc.tile_pool(name="const", bufs=1) as cpool,
            tc.tile_pool(name="gath", bufs=5) as gpool,
            tc.tile_pool(name="work", bufs=7) as wpool,
            tc.tile_pool(name="ph", bufs=3, space="PSUM") as ph_pool,
            tc.tile_pool(name="pm", bufs=3, space="PSUM") as pm_pool,
            tc.tile_pool(name="pa", bufs=2, space="PSUM") as pa_pool,
        ):
            def cload(dram, shape, dtype=f32):
                t = cpool.tile(shape, dtype, name=dram.name + "_t")
                nc.sync.dma_start(out=t[:], in_=dram[:])
                return t

            # critical consts first (needed by the first edge groups)
            mw1_sd_t = cload(mw1_sd_d, [2 * DIN, 2 * DOUT], f8)
            mw1_r_t = cload(mw1_r_d, [NB, 2 * DOUT], f8)
            mb1_t = cload(mb1_d, [2 * DOUT, 1])
            w2u_t = cload(w2u_d, [2 * DOUT, DOUT], bf)

            # pipeline state
            st = {}      # k -> dict(ph, hT, pm, msg)
            chunks = {}  # c -> (xc4, rb4, oh4)
            pa_cur = {}
            CH = 4 * 4 * P  # edges per DMA chunk (4 groups)
            nch = ngrp // 4

            def stage_dma(c):
                e0 = c * CH
                xc4 = gpool.tile([P, CH], f8, tag="xc", name=f"xc_{c}")
                nc.sync.dma_start(out=xc4[:], in_=xcatT_d[:, e0:e0 + CH])
                oh4 = gpool.tile([P, CH], f8, tag="oh", name=f"oh_{c}")
                nc.sync.dma_start(out=oh4[:], in_=ohsT_d[:, e0:e0 + CH])
                rb4 = gpool.tile([NB, CH], f8, tag="rb", name=f"rb_{c}")
                nc.sync.dma_start(out=rb4[:], in_=rbfT_d[:, e0:e0 + CH])
                chunks[c] = (xc4, rb4, oh4)

            stage_dma(0)
            stage_dma(1)

            # secondary consts (flushes / end phase)
            fmul_t = cload(fmul_d, [DOUT, npad])
            u1x_t = cload(u1x_d, [DOUT, npad])
            uw2_t = cload(uw2_d, [DOUT, DOUT], bf)
            ub2_t = cload(ub2_d, [DOUT, 1])
            lng_t = cload(lng_d, [P, DOUT])
            lnb_t = cload(lnb_d, [P, DOUT])
            identf_t = cload(identf_d, [P, P])
            eps_t = cpool.tile([P, 1], f32, name="eps_t")
            nc.vector.memset(eps_t[:], 1e-5)

            # u1 hidden activations for all local nodes, filled per window
            u1_sb = cpool.tile([DOUT, npad], bf, name="u1_sb")

            def stage_l1mm(k):
                s = st[k] = {}
                xc4, rb4, _ = chunks[k // 4]
                q = (k % 4) * 4 * P
                ph = s["ph"] = ph_pool.tile([P, 4 * P], f32, tag="ph",
                                            name=f"ph_{k}")
                nc.tensor.matmul(ph[:], mw1_sd_t[:], xc4[:, q:q + 4 * P],
                                 start=True, stop=False)
                nc.tensor.matmul(ph[:], mw1_r_t[:], rb4[:, q:q + 4 * P],
                                 start=False, stop=True)

            def stage_silu(k):
                s = st[k]
                hT = s["hT"] = wpool.tile([P, 4 * P], bf, tag="hT",
                                          name=f"hT_{k}")
                nc.scalar.activation(out=hT[:], in_=s["ph"][:],
                                     func=ACT.Silu, bias=mb1_t[:, 0:1])

            def stage_l2mm(k):
                s = st[k]
                hT = s["hT"]
                pm = s["pm"] = pm_pool.tile([P, 4 * DOUT], f32, tag="pm",
                                            name=f"pm_{k}")
                for j in range(4):
                    nc.tensor.matmul(pm[:, j * DOUT:(j + 1) * DOUT],
                                     hT[:, j * P:(j + 1) * P],
                                     w2u_t[:], start=True, stop=True)

            def stage_copy(k):
                s = st[k]
                msg = s["msg"] = wpool.tile([P, 4 * DOUT], f8, tag="msg",
                                            name=f"msg_{k}")
                nc.vector.tensor_copy(out=msg[:], in_=s["pm"][:])

            def stage_scatter(k):
                s = st[k]
                msg = s["msg"]
                oh4 = chunks[k // 4][2]
                q = (k % 4) * 4 * P
                for j in range(4):
                    g = 4 * k + j
                    w = block_window[g]
                    if g == wfirst[w]:
                        pa_cur[w] = pa_pool.tile([DOUT, P], f32, tag="pa",
                                                 name=f"pa_w{w}")
                    nc.tensor.matmul(
                        pa_cur[w][:],
                        msg[:, j * DOUT:(j + 1) * DOUT],
                        oh4[:, q + j * P:q + (j + 1) * P],
                        start=(g == wfirst[w]), stop=(g == wlast[w]),
                        skip_group_check=True)
                    if g != wlast[w]:
                        continue
                    wc = slice(w * P, (w + 1) * P)
                    fl = wpool.tile([DOUT, P], f32, tag="fl", name=f"fl_{w}")
                    nc.vector.tensor_tensor(out=fl[:], in0=pa_cur[w][:],
                                            in1=fmul_t[:, wc], op=OP.mult)
                    nc.vector.tensor_tensor(out=fl[:], in0=fl[:],
                                            in1=u1x_t[:, wc], op=OP.add)
                    nc.scalar.activation(out=u1_sb[:, wc], in_=fl[:],
                                         func=ACT.Silu)
                    del pa_cur[w]
                del st[k]

            for k in range(ngrp + 7):
                if 3 <= k < ngrp + 3:
                    stage_l2mm(k - 3)
                if 1 <= k < ngrp + 1:
                    stage_silu(k - 1)
                if k >= 7:
                    stage_scatter(k - 7)
                if 5 <= k < ngrp + 5:
                    stage_copy(k - 5)
                if k < ngrp:
                    if k % 4 == 0 and k // 4 + 2 < nch:
                        stage_dma(k // 4 + 2)
                    stage_l1mm(k)

            # ---------- update layer 2 + LayerNorm (pipelined end phase) ---
            ust = {}
            nug = (npad + 511) // 512

            def stage_upB(m):
                u0 = m * 512
                cw = min(512, npad - u0)
                s = ust[m] = {"cw": cw, "u0": u0}
                pz = s["pz"] = ph_pool.tile([P, 512], f32, tag="ph",
                                            name=f"pz_{m}")
                nc.tensor.matmul(pz[0:DOUT, 0:cw], uw2_t[:],
                                 u1_sb[:, u0:u0 + cw], start=True, stop=True)
                zT = s["zT"] = wpool.tile([DOUT, 512], f32, tag="zT",
                                          name=f"zT_{m}")
                nc.scalar.activation(out=zT[:, 0:cw], in_=pz[0:DOUT, 0:cw],
                                     func=ACT.Identity, bias=ub2_t[:, 0:1])

            def stage_upC(m):
                s = ust[m]
                cw, u0 = s["cw"], s["u0"]
                nj = cw // P
                zT = s["zT"]
                pz2 = pm_pool.tile([P, 4 * DOUT], f32, tag="pm",
                                   name=f"pz2_{m}")
                for j in range(nj):
                    nc.tensor.transpose(
                        out=pz2[:, j * DOUT:(j + 1) * DOUT],
                        in_=zT[:, j * P:(j + 1) * P],
                        identity=identf_t[0:DOUT, 0:DOUT])
                # LayerNorm on [128, nj, 64] (free-axis per-node)
                zc = wpool.tile([P, 4 * DOUT], f32, tag="zc", name=f"zc_{m}")
                red = wpool.tile([P, 4], f32, tag="red", name=f"red_{m}")
                red2 = wpool.tile([P, 4], f32, tag="red2", name=f"red2_{m}")
                z3 = pz2[:, 0:nj * DOUT].rearrange("p (j d) -> p j d", d=DOUT)
                nc.vector.tensor_reduce(out=red[:, 0:nj], in_=z3, axis=AX.X,
                                        op=OP.add)
                nc.vector.tensor_scalar_mul(red[:, 0:nj], red[:, 0:nj],
                                            -1.0 / DOUT)
                zc3 = zc[:, 0:nj * DOUT].rearrange("p (j d) -> p j d", d=DOUT)
                nc.vector.tensor_tensor(
                    out=zc3, in0=z3,
                    in1=red[:, 0:nj, None].to_broadcast([P, nj, DOUT]),
                    op=OP.add)
                sq = wpool.tile([P, 4 * DOUT], f32, tag="sq", name=f"sq_{m}")
                sq3 = sq[:, 0:nj * DOUT].rearrange("p (j d) -> p j d", d=DOUT)
                nc.vector.tensor_tensor(out=sq3, in0=zc3, in1=zc3, op=OP.mult)
                nc.vector.tensor_reduce(out=red2[:, 0:nj], in_=sq3, axis=AX.X,
                                        op=OP.add)
                sd = wpool.tile([P, 4], f32, tag="sd", name=f"sd_{m}")
                nc.scalar.activation(out=sd[:, 0:nj], in_=red2[:, 0:nj],
                                     func=ACT.Sqrt, scale=1.0 / DOUT,
                                     bias=eps_t[:, 0:1])
                rs = wpool.tile([P, 4], f32, tag="rs", name=f"rs_{m}")
                nc.vector.reciprocal(out=rs[:, 0:nj], in_=sd[:, 0:nj])
                zn = wpool.tile([P, 4 * DOUT], f32, tag="zn", name=f"zn_{m}")
                zn3 = zn[:, 0:nj * DOUT].rearrange("p (j d) -> p j d", d=DOUT)
                nc.vector.tensor_tensor(
                    out=zn3, in0=zc3,
                    in1=rs[:, 0:nj, None].to_broadcast([P, nj, DOUT]),
                    op=OP.mult)
                for j in range(nj):
                    js = slice(j * DOUT, (j + 1) * DOUT)
                    nc.vector.tensor_tensor(out=zn[:, js], in0=zn[:, js],
                                            in1=lng_t[:], op=OP.mult)
                    nc.vector.tensor_tensor(out=zn[:, js], in0=zn[:, js],
                                            in1=lnb_t[:], op=OP.add)
                nc.sync.dma_start(
                    out=out_d[u0:u0 + cw].rearrange("(j p) d -> p j d", p=P),
                    in_=zn[:, 0:nj * DOUT].rearrange("p (j d) -> p j d",
                                                     d=DOUT))
                del ust[m]

            for m in range(nug + 2):
                if m < nug:
                    stage_upB(m)
                if m >= 2:
                    stage_upC(m - 2)

    nc.compile()
    return nc


# ---------------------------------------------------------------------------
# Entry point
# ---------------------------------------------------------------------------

last_results = None


def kernel(x, edge_index, edge_vec, edge_len,
           mw1, mb1, mw2, mb2, uw1, ub1, uw2, ub2, ln_g, ln_b):
    global last_results
    import os
    from concourse.bass_utils import run_bass_kernel_spmd

    struct, in_maps = _build_host_data(
        x, edge_index, edge_len, mw1, mb1, mw2, mb2,
        uw1, ub1, uw2, ub2, ln_g, ln_b)

    key = (struct["n"], struct["btot"], struct["bws"])
    if key not in _prog_cache:
        _prog_cache[key] = _build_program(struct)
    nc = _prog_cache[key]

    kw = {}
    if os.environ.get("K_TRACE", ""):
        import profile_shim
        profile_shim.install()
        kw = dict(trace=True, trace_cores=list(range(NCORES)),
                  tmpdir="/tmp/ntff_out")
    res = run_bass_kernel_spmd(nc, in_maps, core_ids=list(range(NCORES)), **kw)
    last_results = res
    nloc = struct["nloc"]
    out = np.concatenate([res.results[c]["out"][:nloc] for c in range(NCORES)],
                         axis=0)
    return out.astype(np.float32)


# revision 62
# speedup vs baseline: 1.0174x; 1.0174x over previous
"""GNN message-passing layer (EquivariantMPLayer) on 8 Trainium2 NeuronCores.

Sharding: edges are sharded by destination-node range (dst // (N/8)) so each
core aggregates its own node range locally -- no collectives needed. Per core,
edges are sorted by dst and grouped into 128-node windows; each window's edge
list is padded to 128-edge blocks. Per-window block counts are equalized
across cores (max over cores) so a single SPMD program serves all 8 cores.

The host pre-gathers x[src] and x[dst] for every edge slot into a single
feature-major stream xcatT [128, epad] (rows 0:64 = src feats, 64:128 = dst
feats, columns in device consumption order), and pre-builds the per-block
scatter one-hots as a second stream ohsT [128, epad] whose nonzero VALUES are
1/cnt(dst) -- so the scatter matmul directly produces the mean aggregate.

Algebraic folds: the L2 weight is W2U = mw2 @ uw1_agg, so the scatter PSUM
accumulates uw1_agg^T @ (agg/cnt) -- the update-MLP's aggregation term --
directly. When a window closes, one extra matmul adds uw1_x^T @ x (with an
augmented constant row supplying has*uw1_agg^T@mb2 exactly), and a single
ACT Silu produces the u1 hidden vector for those 128 nodes. The second
update layer + LayerNorm run as a short pipelined end phase.

The device pipeline is software-pipelined so the tensor engine never waits:
at iteration k it runs L1(k), L2(k-2) and scatter(k-6), while ACT runs
silu(k) and the per-window u1 activation, and DVE only copies msg out of
PSUM.
"""

import numpy as np

N = 50000
E = 800000
DIN = 64
DOUT = 64
NB = 16
MAX_RADIUS = 10.0
NCORES = 8
P = 128

_prog_cache = {}


# ---------------------------------------------------------------------------
# Host-side structure / metadata
# ---------------------------------------------------------------------------

def _build_host_data(x, edge_index, edge_len, mw1, mb1, mw2, mb2,
                     uw1, ub1, uw2, ub2, ln_g, ln_b,
                     n=N, ncores=NCORES):
    import ml_dtypes
    bf16 = ml_dtypes.bfloat16

    nloc = n // ncores
    nw = (nloc + P - 1) // P
    npad = nw * P

    src = np.asarray(edge_index[0], dtype=np.int64)
    dst = np.asarray(edge_index[1], dtype=np.int64)
    x = np.asarray(x, dtype=np.float32)
    el = np.asarray(edge_len, dtype=np.float32)[:, 0]

    centers = np.linspace(0.0, MAX_RADIUS, NB, dtype=np.float64)
    width = (centers[1] - centers[0]) * 0.5
    rbf_all = np.exp(-((el[:, None].astype(np.float64) - centers) ** 2)
                     / (2.0 * width ** 2)).astype(np.float32)  # [E, 16]

    core_of = dst // nloc
    per_core = []
    cnt_w = np.zeros((ncores, nw), dtype=np.int64)
    for c in range(ncores):
        eids = np.nonzero(core_of == c)[0]
        dloc = (dst[eids] - c * nloc).astype(np.int64)
        order = np.argsort(dloc, kind="stable")
        eids = eids[order]
        dloc = dloc[order]
        w_of = dloc // P
        cnt_w[c] = np.bincount(w_of, minlength=nw)
        per_core.append((eids, dloc, w_of))

    # per-window block counts, equalized across cores; total padded to x16
    # (16 blocks = one 4-group DMA chunk)
    bws = np.maximum(1, (cnt_w.max(axis=0) + P - 1) // P)  # [nw]
    bws[-1] += (-int(bws.sum())) % 16
    btot = int(bws.sum())
    epad = btot * P

    block_window = np.repeat(np.arange(nw), bws)
    boff = np.concatenate([[0], np.cumsum(bws)[:-1]])  # first block of window

    uw1f = np.asarray(uw1, np.float32)
    w2u = (np.asarray(mw2, np.float32) @ uw1f[DIN:]).astype(bf16)  # [128, 64]
    mb2u = np.asarray(mb2, np.float32) @ uw1f[DIN:]                # [64]

    in_maps = []
    for c in range(ncores):
        eids, dloc, w_of = per_core[c]
        # slot index for each edge: window base + position within window
        win_start = np.concatenate([[0], np.cumsum(cnt_w[c])[:-1]])
        pos_in_w = np.arange(len(eids)) - win_start[w_of]
        slot = boff[w_of] * P + pos_in_w  # [e_c]

        f8 = ml_dtypes.float8_e4m3

        xcat = np.zeros((epad, 2 * DIN), dtype=np.float32)
        xcat[slot, :DIN] = x[src[eids]]
        xcat[slot, DIN:] = x[dst[eids]]
        xcat8 = np.ascontiguousarray(xcat.T).astype(f8)

        rbf = np.zeros((epad, NB), dtype=np.float32)
        rbf[slot] = rbf_all[eids]
        rbf8 = np.ascontiguousarray(rbf.T).astype(f8)

        cnt_n = np.zeros(npad, dtype=np.float32)
        cnt_n[:nloc] = np.bincount(dloc, minlength=nloc).astype(np.float32)
        inv8 = (1.0 / np.maximum(cnt_n, 1.0)).astype(f8).view(np.uint8)
        has = (cnt_n > 0).astype(np.float32)

        # one-hot stream with fp8 1/cnt values: ohsT[lane, g*P + n] =
        # 1/cnt(node) iff edge slot g*P+lane scatters into window-rel node n
        oh8 = np.zeros((epad, P), dtype=np.uint8)
        oh8[slot, (dloc - w_of * P)] = inv8[dloc]
        ohs8 = np.ascontiguousarray(
            oh8.reshape(btot, P, P).transpose(1, 0, 2).reshape(P, epad)
        ).view(f8)

        # host-precomputed x-side of u1 pre-activation (+ mb2/ub1 folds)
        xt_loc = np.zeros((DIN, npad), dtype=np.float32)
        xt_loc[:, :nloc] = x[c * nloc:(c + 1) * nloc].T
        u1x = (uw1f[:DIN].T @ xt_loc
               + np.outer(mb2u, has)
               + np.asarray(ub1, np.float32)[:, None])

        m = {
            "xcatT": xcat8,
            "rbfT": rbf8,
            "ohsT": ohs8,
            "u1x": u1x,
            "mw1_sd": np.asarray(mw1, np.float32)[:2 * DIN].astype(f8),
            "mw1_r": np.asarray(mw1, np.float32)[2 * DIN:].astype(f8),
            "mb1": np.asarray(mb1, np.float32).reshape(2 * DOUT, 1).copy(),
            "w2u": w2u,
            "uw2": np.asarray(uw2, np.float32).astype(bf16),
            "ub2": np.asarray(ub2, np.float32).reshape(DOUT, 1).copy(),
            "lng": np.broadcast_to(np.asarray(ln_g, np.float32)[None, :],
                                   (P, DOUT)).copy(),
            "lnb": np.broadcast_to(np.asarray(ln_b, np.float32)[None, :],
                                   (P, DOUT)).copy(),
            "identf": np.eye(P, dtype=np.float32),
        }
        in_maps.append(m)

    struct = dict(n=n, nloc=nloc, nw=nw, npad=npad, btot=btot, epad=epad,
                  bws=tuple(int(v) for v in bws),
                  block_window=tuple(int(v) for v in block_window))
    return struct, in_maps


# ---------------------------------------------------------------------------
# Device program
# ---------------------------------------------------------------------------

def _build_program(struct):
    import concourse.bass as bass
    import concourse.mybir as mybir
    import concourse.tile as tile
    from concourse import bacc

    f32 = mybir.dt.float32
    bf = mybir.dt.bfloat16
    f8 = mybir.dt.float8e4
    n, nloc, nw, npad = (struct["n"], struct["nloc"], struct["nw"],
                         struct["npad"])
    btot, epad = struct["btot"], struct["epad"]
    block_window = struct["block_window"]
    ngrp = btot // 4

    # first/last block of each window
    wfirst = {}
    wlast = {}
    for g, w in enumerate(block_window):
        wfirst.setdefault(w, g)
        wlast[w] = g

    nc = bacc.Bacc("TRN2", target_bir_lowering=False, debug=False,
                   enable_asserts=False, num_devices=NCORES)

    xcatT_d = nc.dram_tensor("xcatT", [P, epad], f8, kind="ExternalInput")
    rbfT_d = nc.dram_tensor("rbfT", [NB, epad], f8, kind="ExternalInput")
    ohsT_d = nc.dram_tensor("ohsT", [P, epad], f8, kind="ExternalInput")
    u1x_d = nc.dram_tensor("u1x", [DOUT, npad], f32, kind="ExternalInput")
    mw1_sd_d = nc.dram_tensor("mw1_sd", [2 * DIN, 2 * DOUT], f8,
                              kind="ExternalInput")
    mw1_r_d = nc.dram_tensor("mw1_r", [NB, 2 * DOUT], f8,
                             kind="ExternalInput")
    mb1_d = nc.dram_tensor("mb1", [2 * DOUT, 1], f32, kind="ExternalInput")
    w2u_d = nc.dram_tensor("w2u", [2 * DOUT, DOUT], bf, kind="ExternalInput")
    uw2_d = nc.dram_tensor("uw2", [DOUT, DOUT], bf, kind="ExternalInput")
    ub2_d = nc.dram_tensor("ub2", [DOUT, 1], f32, kind="ExternalInput")
    lng_d = nc.dram_tensor("lng", [P, DOUT], f32, kind="ExternalInput")
    lnb_d = nc.dram_tensor("lnb", [P, DOUT], f32, kind="ExternalInput")
    identf_d = nc.dram_tensor("identf", [P, P], f32, kind="ExternalInput")
    out_d = nc.dram_tensor("out", [npad, DOUT], f32, kind="ExternalOutput")

    AX = mybir.AxisListType
    OP = mybir.AluOpType
    ACT = mybir.ActivationFunctionType

    with tile.TileContext(nc) as tc:
        with (
            tc.tile_pool(name="const", bufs=1) as cpool,
            tc.tile_pool(name="gath", bufs=5) as gpool,
            tc.tile_pool(name="work", bufs=7) as wpool,
            tc.tile_pool(name="ph", bufs=3, space="PSUM") as ph_pool,
            tc.tile_pool(name="pm", bufs=3, space="PSUM") as pm_pool,
            tc.tile_pool(name="pa", bufs=2, space="PSUM") as pa_pool,
        ):
            def cload(dram, shape, dtype=f32):
                t = cpool.tile(shape, dtype, name=dram.name + "_t")
                nc.sync.dma_start(out=t[:], in_=dram[:])
                return t

            # critical consts first (needed by the first edge groups)
            mw1_sd_t = cload(mw1_sd_d, [2 * DIN, 2 * DOUT], f8)
            mw1_r_t = cload(mw1_r_d, [NB, 2 * DOUT], f8)
            mb1_t = cload(mb1_d, [2 * DOUT, 1])
            w2u_t = cload(w2u_d, [2 * DOUT, DOUT], bf)

            # pipeline state
            st = {}      # k -> dict(ph, hT, pm, msg)
            chunks = {}  # c -> (xc4, rb4, oh4)
            pa_cur = {}
            closed = []  # windows whose PSUM group closed, flush pending
            CH = 4 * 4 * P  # edges per DMA chunk (4 groups)
            nch = ngrp // 4

            def stage_dma(c):
                e0 = c * CH
                xc4 = gpool.tile([P, CH], f8, tag="xc", name=f"xc_{c}")
                nc.sync.dma_start(out=xc4[:], in_=xcatT_d[:, e0:e0 + CH])
                oh4 = gpool.tile([P, CH], f8, tag="oh", name=f"oh_{c}")
                nc.sync.dma_start(out=oh4[:], in_=ohsT_d[:, e0:e0 + CH])
                rb4 = gpool.tile([NB, CH], f8, tag="rb", name=f"rb_{c}")
                nc.sync.dma_start(out=rb4[:], in_=rbfT_d[:, e0:e0 + CH])
                chunks[c] = (xc4, rb4, oh4)

            stage_dma(0)
            stage_dma(1)

            # secondary consts (flushes / end phase)
            u1x_t = cload(u1x_d, [DOUT, npad])
            uw2_t = cload(uw2_d, [DOUT, DOUT], bf)
            ub2_t = cload(ub2_d, [DOUT, 1])
            lng_t = cload(lng_d, [P, DOUT])
            lnb_t = cload(lnb_d, [P, DOUT])
            identf_t = cload(identf_d, [P, P])
            eps_t = cpool.tile([P, 1], f32, name="eps_t")
            nc.vector.memset(eps_t[:], 1e-5)

            # u1 hidden activations for all local nodes, filled per window
            u1_sb = cpool.tile([DOUT, npad], bf, name="u1_sb")

            def stage_l1mm(k):
                s = st[k] = {}
                xc4, rb4, _ = chunks[k // 4]
                q = (k % 4) * 4 * P
                ph = s["ph"] = ph_pool.tile([P, 4 * P], f32, tag="ph",
                                            name=f"ph_{k}")
                nc.tensor.matmul(ph[:], mw1_sd_t[:], xc4[:, q:q + 4 * P],
                                 start=True, stop=False)
                nc.tensor.matmul(ph[:], mw1_r_t[:], rb4[:, q:q + 4 * P],
                                 start=False, stop=True)

            def stage_silu(k):
                s = st[k]
                hT = s["hT"] = wpool.tile([P, 4 * P], bf, tag="hT",
                                          name=f"hT_{k}")
                nc.scalar.activation(out=hT[:], in_=s["ph"][:],
                                     func=ACT.Silu, bias=mb1_t[:, 0:1])

            def stage_l2mm(k):
                s = st[k]
                hT = s["hT"]
                pm = s["pm"] = pm_pool.tile([P, 4 * DOUT], f32, tag="pm",
                                            name=f"pm_{k}")
                for j in range(4):
                    nc.tensor.matmul(pm[:, j * DOUT:(j + 1) * DOUT],
                                     hT[:, j * P:(j + 1) * P],
                                     w2u_t[:], start=True, stop=True)

            def stage_copy(k):
                s = st[k]
                msg = s["msg"] = wpool.tile([P, 4 * DOUT], f8, tag="msg",
                                            name=f"msg_{k}")
                nc.vector.tensor_copy(out=msg[:], in_=s["pm"][:])

            def stage_scatter(k):
                s = st[k]
                msg = s["msg"]
                oh4 = chunks[k // 4][2]
                q = (k % 4) * 4 * P
                for j in range(4):
                    g = 4 * k + j
                    w = block_window[g]
                    if g == wfirst[w]:
                        pa_cur[w] = pa_pool.tile([DOUT, P], f32, tag="pa",
                                                 name=f"pa_w{w}")
                    nc.tensor.matmul(
                        pa_cur[w][:],
                        msg[:, j * DOUT:(j + 1) * DOUT],
                        oh4[:, q + j * P:q + (j + 1) * P],
                        start=(g == wfirst[w]), stop=(g == wlast[w]),
                        skip_group_check=True)
                    if g != wlast[w]:
                        continue
                    closed.append((w, pa_cur.pop(w)))
                del st[k]

            def stage_flush():
                for w, pa in closed:
                    wc = slice(w * P, (w + 1) * P)
                    fl = wpool.tile([DOUT, P], f32, tag="fl", name=f"fl_{w}")
                    nc.vector.tensor_tensor(out=fl[:], in0=pa[:],
                                            in1=u1x_t[:, wc], op=OP.add)
                    nc.scalar.activation(out=u1_sb[:, wc], in_=fl[:],
                                         func=ACT.Silu)
                closed.clear()

            for k in range(ngrp + 7):
                if 3 <= k < ngrp + 3:
                    stage_l2mm(k - 3)
                if 1 <= k < ngrp + 1:
                    stage_silu(k - 1)
                if k >= 7:
                    stage_scatter(k - 7)
                if 5 <= k < ngrp + 5:
                    stage_copy(k - 5)
                stage_flush()
                if k < ngrp:
                    if k % 4 == 0 and k // 4 + 2 < nch:
                        stage_dma(k // 4 + 2)
                    stage_l1mm(k)

            # ---------- update layer 2 + LayerNorm (pipelined end phase) ---
            ust = {}
            nug = (npad + 511) // 512

            def stage_upB(m):
                u0 = m * 512
                cw = min(512, npad - u0)
                s = ust[m] = {"cw": cw, "u0": u0}
                pz = s["pz"] = ph_pool.tile([P, 512], f32, tag="ph",
                                            name=f"pz_{m}")
                nc.tensor.matmul(pz[0:DOUT, 0:cw], uw2_t[:],
                                 u1_sb[:, u0:u0 + cw], start=True, stop=True)
                zT = s["zT"] = wpool.tile([DOUT, 512], f32, tag="zT",
                                          name=f"zT_{m}")
                nc.scalar.activation(out=zT[:, 0:cw], in_=pz[0:DOUT, 0:cw],
                                     func=ACT.Identity, bias=ub2_t[:, 0:1])

            def stage_upC(m):
                s = ust[m]
                cw, u0 = s["cw"], s["u0"]
                nj = cw // P
                zT = s["zT"]
                pz2 = pm_pool.tile([P, 4 * DOUT], f32, tag="pm",
                                   name=f"pz2_{m}")
                for j in range(nj):
                    nc.tensor.transpose(
                        out=pz2[:, j * DOUT:(j + 1) * DOUT],
                        in_=zT[:, j * P:(j + 1) * P],
                        identity=identf_t[0:DOUT, 0:DOUT])
                # LayerNorm on [128, nj, 64] (free-axis per-node)
                zc = wpool.tile([P, 4 * DOUT], f32, tag="zc", name=f"zc_{m}")
                red = wpool.tile([P, 4], f32, tag="red", name=f"red_{m}")
                red2 = wpool.tile([P, 4], f32, tag="red2", name=f"red2_{m}")
                z3 = pz2[:, 0:nj * DOUT].rearrange("p (j d) -> p j d", d=DOUT)
                nc.vector.tensor_reduce(out=red[:, 0:nj], in_=z3, axis=AX.X,
                                        op=OP.add)
                nc.vector.tensor_scalar_mul(red[:, 0:nj], red[:, 0:nj],
                                            -1.0 / DOUT)
                zc3 = zc[:, 0:nj * DOUT].rearrange("p (j d) -> p j d", d=DOUT)
                nc.vector.tensor_tensor(
                    out=zc3, in0=z3,
                    in1=red[:, 0:nj, None].to_broadcast([P, nj, DOUT]),
                    op=OP.add)
                sq = wpool.tile([P, 4 * DOUT], f32, tag="sq", name=f"sq_{m}")
                sq3 = sq[:, 0:nj * DOUT].rearrange("p (j d) -> p j d", d=DOUT)
                nc.vector.tensor_tensor(out=sq3, in0=zc3, in1=zc3, op=OP.mult)
                nc.vector.tensor_reduce(out=red2[:, 0:nj], in_=sq3, axis=AX.X,
                                        op=OP.add)
                sd = wpool.tile([P, 4], f32, tag="sd", name=f"sd_{m}")
                nc.scalar.activation(out=sd[:, 0:nj], in_=red2[:, 0:nj],
                                     func=ACT.Sqrt, scale=1.0 / DOUT,
                                     bias=eps_t[:, 0:1])
                rs = wpool.tile([P, 4], f32, tag="rs", name=f"rs_{m}")
                nc.vector.reciprocal(out=rs[:, 0:nj], in_=sd[:, 0:nj])
                zn = wpool.tile([P, 4 * DOUT], f32, tag="zn", name=f"zn_{m}")
                zn3 = zn[:, 0:nj * DOUT].rearrange("p (j d) -> p j d", d=DOUT)
                nc.vector.tensor_tensor(
                    out=zn3, in0=zc3,
                    in1=rs[:, 0:nj, None].to_broadcast([P, nj, DOUT]),
                    op=OP.mult)
                for j in range(nj):
                    js = slice(j * DOUT, (j + 1) * DOUT)
                    nc.vector.tensor_tensor(out=zn[:, js], in0=zn[:, js],
                                            in1=lng_t[:], op=OP.mult)
                    nc.vector.tensor_tensor(out=zn[:, js], in0=zn[:, js],
                                            in1=lnb_t[:], op=OP.add)
                nc.sync.dma_start(
                    out=out_d[u0:u0 + cw].rearrange("(j p) d -> p j d", p=P),
                    in_=zn[:, 0:nj * DOUT].rearrange("p (j d) -> p j d",
                                                     d=DOUT))
                del ust[m]

            for m in range(nug + 2):
                if m < nug:
                    stage_upB(m)
                if m >= 2:
                    stage_upC(m - 2)

    nc.compile()
    return nc


# ---------------------------------------------------------------------------
# Entry point
# ---------------------------------------------------------------------------

last_results = None


def kernel(x, edge_index, edge_vec, edge_len,
           mw1, mb1, mw2, mb2, uw1, ub1, uw2, ub2, ln_g, ln_b):
    global last_results
    import os
    from concourse.bass_utils import run_bass_kernel_spmd

    struct, in_maps = _build_host_data(
        x, edge_index, edge_len, mw1, mb1, mw2, mb2,
        uw1, ub1, uw2, ub2, ln_g, ln_b)

    key = (struct["n"], struct["btot"], struct["bws"])
    if key not in _prog_cache:
        _prog_cache[key] = _build_program(struct)
    nc = _prog_cache[key]

    kw = {}
    if os.environ.get("K_TRACE", ""):
        import profile_shim
        profile_shim.install()
        kw = dict(trace=True, trace_cores=list(range(NCORES)),
                  tmpdir="/tmp/ntff_out")
    res = run_bass_kernel_spmd(nc, in_maps, core_ids=list(range(NCORES)), **kw)
    last_results = res
    nloc = struct["nloc"]
    out = np.concatenate([res.results[c]["out"][:nloc] for c in range(NCORES)],
                         axis=0)
    return out.astype(np.float32)


# revision 71
# speedup vs baseline: 1.3328x; 1.3100x over previous
"""GNN message-passing layer (EquivariantMPLayer) on 8 Trainium2 NeuronCores.

Sharding: edges are sharded by destination-node range (dst // (N/8)) so each
core aggregates its own node range locally -- no collectives needed. Per core,
edges are sorted by dst and grouped into 128-node windows; each window's edge
list is padded to 128-edge blocks. Per-window block counts are equalized
across cores (max over cores) so a single SPMD program serves all 8 cores.

The host pre-gathers x[src] and x[dst] for every edge slot into a single
feature-major stream xcatT [128, epad] (rows 0:64 = src feats, 64:128 = dst
feats, columns in device consumption order), and pre-builds the per-block
scatter one-hots as a second stream ohsT [128, epad] whose nonzero VALUES are
1/cnt(dst) -- so the scatter matmul directly produces the mean aggregate.

Algebraic folds: the L2 weight is W2U = mw2 @ uw1_agg, so the scatter PSUM
accumulates uw1_agg^T @ (agg/cnt) -- the update-MLP's aggregation term --
directly. When a window closes, one extra matmul adds uw1_x^T @ x (with an
augmented constant row supplying has*uw1_agg^T@mb2 exactly), and a single
ACT Silu produces the u1 hidden vector for those 128 nodes. The second
update layer + LayerNorm run as a short pipelined end phase.

The device pipeline is software-pipelined so the tensor engine never waits:
at iteration k it runs L1(k), L2(k-2) and scatter(k-6), while ACT runs
silu(k) and the per-window u1 activation, and DVE only copies msg out of
PSUM.
"""

import numpy as np

N = 50000
E = 800000
DIN = 64
DOUT = 64
NB = 16
MAX_RADIUS = 10.0
NCORES = 8
P = 128

_prog_cache = {}


# ---------------------------------------------------------------------------
# Host-side structure / metadata
# ---------------------------------------------------------------------------

def _build_host_data(x, edge_index, edge_len, mw1, mb1, mw2, mb2,
                     uw1, ub1, uw2, ub2, ln_g, ln_b,
                     n=N, ncores=NCORES):
    import ml_dtypes
    bf16 = ml_dtypes.bfloat16

    nloc = n // ncores
    nw = (nloc + P - 1) // P
    npad = nw * P

    src = np.asarray(edge_index[0], dtype=np.int64)
    dst = np.asarray(edge_index[1], dtype=np.int64)
    x = np.asarray(x, dtype=np.float32)
    el = np.asarray(edge_len, dtype=np.float32)[:, 0]

    centers = np.linspace(0.0, MAX_RADIUS, NB, dtype=np.float64)
    width = (centers[1] - centers[0]) * 0.5
    rbf_all = np.exp(-((el[:, None].astype(np.float64) - centers) ** 2)
                     / (2.0 * width ** 2)).astype(np.float32)  # [E, 16]

    core_of = dst // nloc
    per_core = []
    cnt_w = np.zeros((ncores, nw), dtype=np.int64)
    for c in range(ncores):
        eids = np.nonzero(core_of == c)[0]
        dloc = (dst[eids] - c * nloc).astype(np.int64)
        order = np.argsort(dloc, kind="stable")
        eids = eids[order]
        dloc = dloc[order]
        w_of = dloc // P
        cnt_w[c] = np.bincount(w_of, minlength=nw)
        per_core.append((eids, dloc, w_of))

    # per-window block counts, equalized across cores; total padded to x16
    # (16 blocks = one 4-group DMA chunk)
    bws = np.maximum(1, (cnt_w.max(axis=0) + P - 1) // P)  # [nw]
    bws[-1] += (-int(bws.sum())) % 16
    btot = int(bws.sum())
    epad = btot * P

    block_window = np.repeat(np.arange(nw), bws)
    boff = np.concatenate([[0], np.cumsum(bws)[:-1]])  # first block of window

    uw1f = np.asarray(uw1, np.float32)
    w2u = (np.asarray(mw2, np.float32) @ uw1f[DIN:]).astype(bf16)  # [128, 64]
    mb2u = np.asarray(mb2, np.float32) @ uw1f[DIN:]                # [64]

    in_maps = []
    for c in range(ncores):
        eids, dloc, w_of = per_core[c]
        # slot index for each edge: window base + position within window
        win_start = np.concatenate([[0], np.cumsum(cnt_w[c])[:-1]])
        pos_in_w = np.arange(len(eids)) - win_start[w_of]
        slot = boff[w_of] * P + pos_in_w  # [e_c]

        f8 = ml_dtypes.float8_e4m3

        xcat = np.zeros((epad, 2 * DIN), dtype=np.float32)
        xcat[slot, :DIN] = x[src[eids]]
        xcat[slot, DIN:] = x[dst[eids]]
        xcat8 = np.ascontiguousarray(xcat.T).astype(f8)

        rbf = np.zeros((epad, NB), dtype=np.float32)
        rbf[slot] = rbf_all[eids]
        rbf8 = np.ascontiguousarray(rbf.T).astype(f8)

        cnt_n = np.zeros(npad, dtype=np.float32)
        cnt_n[:nloc] = np.bincount(dloc, minlength=nloc).astype(np.float32)
        inv = (1.0 / np.maximum(cnt_n, 1.0)).astype(bf16)
        has = (cnt_n > 0).astype(np.float32)

        # one-hot stream with bf16 1/cnt values: ohsT[lane, g*P + n] =
        # 1/cnt(node) iff edge slot g*P+lane scatters into window-rel node n
        inv16 = inv.view(np.uint16)
        oh16 = np.zeros((epad, P), dtype=np.uint16)
        oh16[slot, (dloc - w_of * P)] = inv16[dloc]
        ohsT = np.ascontiguousarray(
            oh16.reshape(btot, P, P).transpose(1, 0, 2).reshape(P, epad)
        ).view(bf16)

        # augmented x: rows 0:64 = x^T, row 64 = has (carries mb2u per node)
        xtaug = np.zeros((DIN + 1, npad), dtype=np.float32)
        xtaug[:DIN, :nloc] = x[c * nloc:(c + 1) * nloc].T
        xtaug[DIN] = has
        uw1x_aug = np.concatenate(
            [uw1f[:DIN], mb2u[None, :]], axis=0)  # [65, 64]

        m = {
            "xcatT": xcat8,
            "rbfT": rbf8,
            "ohsT": ohsT,
            "xtaug": xtaug.astype(bf16),
            "uw1x": uw1x_aug.astype(bf16),
            "ub1": np.asarray(ub1, np.float32).reshape(DOUT, 1).copy(),
            "mw1_sd": np.asarray(mw1, np.float32)[:2 * DIN].astype(f8),
            "mw1_r": np.asarray(mw1, np.float32)[2 * DIN:].astype(f8),
            "mb1": np.asarray(mb1, np.float32).reshape(2 * DOUT, 1).copy(),
            "w2u": w2u,
            "uw2": np.asarray(uw2, np.float32).astype(bf16),
            "ub2": np.asarray(ub2, np.float32).reshape(DOUT, 1).copy(),
            "lng": np.broadcast_to(np.asarray(ln_g, np.float32)[None, :],
                                   (P, DOUT)).copy(),
            "lnb": np.broadcast_to(np.asarray(ln_b, np.float32)[None, :],
                                   (P, DOUT)).copy(),
            "identf": np.eye(P, dtype=np.float32),
        }
        in_maps.append(m)

    struct = dict(n=n, nloc=nloc, nw=nw, npad=npad, btot=btot, epad=epad,
                  bws=tuple(int(v) for v in bws),
                  block_window=tuple(int(v) for v in block_window))
    return struct, in_maps


# ---------------------------------------------------------------------------
# Device program
# ---------------------------------------------------------------------------

def _build_program(struct):
    import concourse.bass as bass
    import concourse.mybir as mybir
    import concourse.tile as tile
    from concourse import bacc

    f32 = mybir.dt.float32
    bf = mybir.dt.bfloat16
    f8 = mybir.dt.float8e4
    n, nloc, nw, npad = (struct["n"], struct["nloc"], struct["nw"],
                         struct["npad"])
    btot, epad = struct["btot"], struct["epad"]
    block_window = struct["block_window"]
    ngrp = btot // 4

    # first/last block of each window
    wfirst = {}
    wlast = {}
    for g, w in enumerate(block_window):
        wfirst.setdefault(w, g)
        wlast[w] = g

    nc = bacc.Bacc("TRN2", target_bir_lowering=False, debug=False,
                   enable_asserts=False, num_devices=NCORES)

    xcatT_d = nc.dram_tensor("xcatT", [P, epad], f8, kind="ExternalInput")
    rbfT_d = nc.dram_tensor("rbfT", [NB, epad], f8, kind="ExternalInput")
    ohsT_d = nc.dram_tensor("ohsT", [P, epad], bf, kind="ExternalInput")
    xtaug_d = nc.dram_tensor("xtaug", [DIN + 1, npad], bf,
                             kind="ExternalInput")
    uw1x_d = nc.dram_tensor("uw1x", [DIN + 1, DOUT], bf,
                            kind="ExternalInput")
    ub1_d = nc.dram_tensor("ub1", [DOUT, 1], f32, kind="ExternalInput")
    mw1_sd_d = nc.dram_tensor("mw1_sd", [2 * DIN, 2 * DOUT], f8,
                              kind="ExternalInput")
    mw1_r_d = nc.dram_tensor("mw1_r", [NB, 2 * DOUT], f8,
                             kind="ExternalInput")
    mb1_d = nc.dram_tensor("mb1", [2 * DOUT, 1], f32, kind="ExternalInput")
    w2u_d = nc.dram_tensor("w2u", [2 * DOUT, DOUT], bf, kind="ExternalInput")
    uw2_d = nc.dram_tensor("uw2", [DOUT, DOUT], bf, kind="ExternalInput")
    ub2_d = nc.dram_tensor("ub2", [DOUT, 1], f32, kind="ExternalInput")
    lng_d = nc.dram_tensor("lng", [P, DOUT], f32, kind="ExternalInput")
    lnb_d = nc.dram_tensor("lnb", [P, DOUT], f32, kind="ExternalInput")
    identf_d = nc.dram_tensor("identf", [P, P], f32, kind="ExternalInput")
    out_d = nc.dram_tensor("out", [npad, DOUT], f32, kind="ExternalOutput")

    AX = mybir.AxisListType
    OP = mybir.AluOpType
    ACT = mybir.ActivationFunctionType

    with tile.TileContext(nc) as tc:
        with (
            tc.tile_pool(name="const", bufs=1) as cpool,
            tc.tile_pool(name="gath", bufs=5) as gpool,
            tc.tile_pool(name="work", bufs=7) as wpool,
            tc.tile_pool(name="ph", bufs=3, space="PSUM") as ph_pool,
            tc.tile_pool(name="pm", bufs=3, space="PSUM") as pm_pool,
            tc.tile_pool(name="pa", bufs=2, space="PSUM") as pa_pool,
        ):
            def cload(dram, shape, dtype=f32):
                t = cpool.tile(shape, dtype, name=dram.name + "_t")
                nc.sync.dma_start(out=t[:], in_=dram[:])
                return t

            # critical consts first (needed by the first edge groups)
            mw1_sd_t = cload(mw1_sd_d, [2 * DIN, 2 * DOUT], f8)
            mw1_r_t = cload(mw1_r_d, [NB, 2 * DOUT], f8)
            mb1_t = cload(mb1_d, [2 * DOUT, 1])
            w2u_t = cload(w2u_d, [2 * DOUT, DOUT], bf)
            uw1x_t = cload(uw1x_d, [DIN + 1, DOUT], bf)
            ub1_t = cload(ub1_d, [DOUT, 1])

            # pipeline state
            st = {}      # k -> dict(ph, hT, pm, msg)
            chunks = {}  # c -> (xc4, rb4, oh4)
            pa_cur = {}
            CH = 4 * 4 * P  # edges per DMA chunk (4 groups)
            nch = ngrp // 4

            def stage_dma(c):
                e0 = c * CH
                xc4 = gpool.tile([P, CH], f8, tag="xc", name=f"xc_{c}")
                nc.sync.dma_start(out=xc4[:], in_=xcatT_d[:, e0:e0 + CH])
                oh4 = gpool.tile([P, CH], bf, tag="oh", name=f"oh_{c}")
                nc.sync.dma_start(out=oh4[:], in_=ohsT_d[:, e0:e0 + CH])
                rb4 = gpool.tile([NB, CH], f8, tag="rb", name=f"rb_{c}")
                nc.sync.dma_start(out=rb4[:], in_=rbfT_d[:, e0:e0 + CH])
                chunks[c] = (xc4, rb4, oh4)

            stage_dma(0)
            stage_dma(1)

            # secondary consts (flushes / end phase)
            xtaug_t = cload(xtaug_d, [DIN + 1, npad], bf)
            uw2_t = cload(uw2_d, [DOUT, DOUT], bf)
            ub2_t = cload(ub2_d, [DOUT, 1])
            lng_t = cload(lng_d, [P, DOUT])
            lnb_t = cload(lnb_d, [P, DOUT])
            identf_t = cload(identf_d, [P, P])
            eps_t = cpool.tile([P, 1], f32, name="eps_t")
            nc.vector.memset(eps_t[:], 1e-5)

            # u1 hidden activations for all local nodes, filled per window
            u1_sb = cpool.tile([DOUT, npad], bf, name="u1_sb")

            def stage_l1mm(k):
                s = st[k] = {}
                xc4, rb4, _ = chunks[k // 4]
                q = (k % 4) * 4 * P
                ph = s["ph"] = ph_pool.tile([P, 4 * P], f32, tag="ph",
                                            name=f"ph_{k}")
                nc.tensor.matmul(ph[:], mw1_sd_t[:], xc4[:, q:q + 4 * P],
                                 start=True, stop=False)
                nc.tensor.matmul(ph[:], mw1_r_t[:], rb4[:, q:q + 4 * P],
                                 start=False, stop=True)

            def stage_silu(k):
                s = st[k]
                hT = s["hT"] = wpool.tile([P, 4 * P], bf, tag="hT",
                                          name=f"hT_{k}")
                nc.scalar.activation(out=hT[:], in_=s["ph"][:],
                                     func=ACT.Silu, bias=mb1_t[:, 0:1])

            def stage_l2mm(k):
                s = st[k]
                hT = s["hT"]
                pm = s["pm"] = pm_pool.tile([P, 4 * DOUT], f32, tag="pm",
                                            name=f"pm_{k}")
                for j in range(4):
                    nc.tensor.matmul(pm[:, j * DOUT:(j + 1) * DOUT],
                                     hT[:, j * P:(j + 1) * P],
                                     w2u_t[:], start=True, stop=True)

            def stage_copy(k):
                s = st[k]
                msg = s["msg"] = wpool.tile([P, 4 * DOUT], bf, tag="msg",
                                            name=f"msg_{k}")
                nc.vector.tensor_copy(out=msg[:], in_=s["pm"][:])

            def stage_scatter(k):
                s = st[k]
                msg = s["msg"]
                oh4 = chunks[k // 4][2]
                q = (k % 4) * 4 * P
                for j in range(4):
                    g = 4 * k + j
                    w = block_window[g]
                    if g == wfirst[w]:
                        pa_cur[w] = pa_pool.tile([DOUT, P], f32, tag="pa",
                                                 name=f"pa_w{w}")
                    nc.tensor.matmul(
                        pa_cur[w][:],
                        msg[:, j * DOUT:(j + 1) * DOUT],
                        oh4[:, q + j * P:q + (j + 1) * P],
                        start=(g == wfirst[w]), stop=False,
                        skip_group_check=True)
                    if g != wlast[w]:
                        continue
                    wc = slice(w * P, (w + 1) * P)
                    # add uw1_x^T @ [x; has] and close the PSUM group
                    nc.tensor.matmul(pa_cur[w][:], uw1x_t[:],
                                     xtaug_t[:, wc], start=False, stop=True,
                                     skip_group_check=True)
                    nc.scalar.activation(out=u1_sb[:, wc], in_=pa_cur[w][:],
                                         func=ACT.Silu, bias=ub1_t[:, 0:1])
                    del pa_cur[w]
                del st[k]

            for k in range(ngrp + 7):
                if 3 <= k < ngrp + 3:
                    stage_l2mm(k - 3)
                if 1 <= k < ngrp + 1:
                    stage_silu(k - 1)
                if k >= 7:
                    stage_scatter(k - 7)
                if 5 <= k < ngrp + 5:
                    stage_copy(k - 5)
                if k < ngrp:
                    if k % 4 == 0 and k // 4 + 2 < nch:
                        stage_dma(k // 4 + 2)
                    stage_l1mm(k)

            # ---------- update layer 2 + LayerNorm (pipelined end phase) ---
            ust = {}
            nug = (npad + 511) // 512

            def stage_upB(m):
                u0 = m * 512
                cw = min(512, npad - u0)
                s = ust[m] = {"cw": cw, "u0": u0}
                pz = s["pz"] = ph_pool.tile([P, 512], f32, tag="ph",
                                            name=f"pz_{m}")
                nc.tensor.matmul(pz[0:DOUT, 0:cw], uw2_t[:],
                                 u1_sb[:, u0:u0 + cw], start=True, stop=True)
                zT = s["zT"] = wpool.tile([DOUT, 512], f32, tag="zT",
                                          name=f"zT_{m}")
                nc.scalar.activation(out=zT[:, 0:cw], in_=pz[0:DOUT, 0:cw],
                                     func=ACT.Identity, bias=ub2_t[:, 0:1])

            def stage_upC(m):
                s = ust[m]
                cw, u0 = s["cw"], s["u0"]
                nj = cw // P
                zT = s["zT"]
                pz2 = pm_pool.tile([P, 4 * DOUT], f32, tag="pm",
                                   name=f"pz2_{m}")
                for j in range(nj):
                    nc.tensor.transpose(
                        out=pz2[:, j * DOUT:(j + 1) * DOUT],
                        in_=zT[:, j * P:(j + 1) * P],
                        identity=identf_t[0:DOUT, 0:DOUT])
                # LayerNorm on [128, nj, 64] (free-axis per-node)
                zc = wpool.tile([P, 4 * DOUT], f32, tag="zc", name=f"zc_{m}")
                red = wpool.tile([P, 4], f32, tag="red", name=f"red_{m}")
                red2 = wpool.tile([P, 4], f32, tag="red2", name=f"red2_{m}")
                z3 = pz2[:, 0:nj * DOUT].rearrange("p (j d) -> p j d", d=DOUT)
                nc.vector.tensor_reduce(out=red[:, 0:nj], in_=z3, axis=AX.X,
                                        op=OP.add)
                nc.vector.tensor_scalar_mul(red[:, 0:nj], red[:, 0:nj],
                                            -1.0 / DOUT)
                zc3 = zc[:, 0:nj * DOUT].rearrange("p (j d) -> p j d", d=DOUT)
                nc.vector.tensor_tensor(
                    out=zc3, in0=z3,
                    in1=red[:, 0:nj, None].to_broadcast([P, nj, DOUT]),
                    op=OP.add)
                sq = wpool.tile([P, 4 * DOUT], f32, tag="sq", name=f"sq_{m}")
                sq3 = sq[:, 0:nj * DOUT].rearrange("p (j d) -> p j d", d=DOUT)
                nc.vector.tensor_tensor(out=sq3, in0=zc3, in1=zc3, op=OP.mult)
                nc.vector.tensor_reduce(out=red2[:, 0:nj], in_=sq3, axis=AX.X,
                                        op=OP.add)
                sd = wpool.tile([P, 4], f32, tag="sd", name=f"sd_{m}")
                nc.scalar.activation(out=sd[:, 0:nj], in_=red2[:, 0:nj],
                                     func=ACT.Sqrt, scale=1.0 / DOUT,
                                     bias=eps_t[:, 0:1])
                rs = wpool.tile([P, 4], f32, tag="rs", name=f"rs_{m}")
                nc.vector.reciprocal(out=rs[:, 0:nj], in_=sd[:, 0:nj])
                zn = wpool.tile([P, 4 * DOUT], f32, tag="zn", name=f"zn_{m}")
                zn3 = zn[:, 0:nj * DOUT].rearrange("p (j d) -> p j d", d=DOUT)
                nc.vector.tensor_tensor(
                    out=zn3, in0=zc3,
                    in1=rs[:, 0:nj, None].to_broadcast([P, nj, DOUT]),
                    op=OP.mult)
                for j in range(nj):
                    js = slice(j * DOUT, (j + 1) * DOUT)
                    nc.vector.tensor_tensor(out=zn[:, js], in0=zn[:, js],
                                            in1=lng_t[:], op=OP.mult)
                    nc.vector.tensor_tensor(out=zn[:, js], in0=zn[:, js],
                                            in1=lnb_t[:], op=OP.add)
                nc.sync.dma_start(
                    out=out_d[u0:u0 + cw].rearrange("(j p) d -> p j d", p=P),
                    in_=zn[:, 0:nj * DOUT].rearrange("p (j d) -> p j d",
                                                     d=DOUT))
                del ust[m]

            for m in range(nug + 2):
                if m < nug:
                    stage_upB(m)
                if m >= 2:
                    stage_upC(m - 2)

    nc.compile()
    return nc


# ---------------------------------------------------------------------------
# Entry point
# ---------------------------------------------------------------------------

last_results = None


def kernel(x, edge_index, edge_vec, edge_len,
           mw1, mb1, mw2, mb2, uw1, ub1, uw2, ub2, ln_g, ln_b):
    global last_results
    import os
    from concourse.bass_utils import run_bass_kernel_spmd

    struct, in_maps = _build_host_data(
        x, edge_index, edge_len, mw1, mb1, mw2, mb2,
        uw1, ub1, uw2, ub2, ln_g, ln_b)

    key = (struct["n"], struct["btot"], struct["bws"])
    if key not in _prog_cache:
        _prog_cache[key] = _build_program(struct)
    nc = _prog_cache[key]

    kw = {}
    if os.environ.get("K_TRACE", ""):
        import profile_shim
        profile_shim.install()
        kw = dict(trace=True, trace_cores=list(range(NCORES)),
                  tmpdir="/tmp/ntff_out")
    res = run_bass_kernel_spmd(nc, in_maps, core_ids=list(range(NCORES)), **kw)
    last_results = res
    nloc = struct["nloc"]
    out = np.concatenate([res.results[c]["out"][:nloc] for c in range(NCORES)],
                         axis=0)
    return out.astype(np.float32)


# revision 77
# speedup vs baseline: 1.4809x; 1.1111x over previous
"""GNN message-passing layer (EquivariantMPLayer) on 8 Trainium2 NeuronCores.

Sharding: edges are sharded by destination-node range (dst // (N/8)) so each
core aggregates its own node range locally -- no collectives needed. Per core,
edges are sorted by dst and grouped into 128-node windows; each window's edge
list is padded to 128-edge blocks. Per-window block counts are equalized
across cores (max over cores) so a single SPMD program serves all 8 cores.

The host pre-gathers x[src] and x[dst] for every edge slot into a single
feature-major stream xcatT [128, epad] (rows 0:64 = src feats, 64:128 = dst
feats, columns in device consumption order), and pre-builds the per-block
scatter one-hots as a second stream ohsT [128, epad] whose nonzero VALUES are
1/cnt(dst) -- so the scatter matmul directly produces the mean aggregate.

Algebraic folds: the L2 weight is W2U = mw2 @ uw1_agg, so the scatter PSUM
accumulates uw1_agg^T @ (agg/cnt) -- the update-MLP's aggregation term --
directly. When a window closes, one extra matmul adds uw1_x^T @ x (with an
augmented constant row supplying has*uw1_agg^T@mb2 exactly), and a single
ACT Silu produces the u1 hidden vector for those 128 nodes. The second
update layer + LayerNorm run as a short pipelined end phase.

The device pipeline is software-pipelined so the tensor engine never waits:
at iteration k it runs L1(k), L2(k-2) and scatter(k-6), while ACT runs
silu(k) and the per-window u1 activation, and DVE only copies msg out of
PSUM.
"""

import numpy as np

N = 50000
E = 800000
DIN = 64
DOUT = 64
NB = 16
MAX_RADIUS = 10.0
NCORES = 8
P = 128

_prog_cache = {}


# ---------------------------------------------------------------------------
# Host-side structure / metadata
# ---------------------------------------------------------------------------

def _build_host_data(x, edge_index, edge_len, mw1, mb1, mw2, mb2,
                     uw1, ub1, uw2, ub2, ln_g, ln_b,
                     n=N, ncores=NCORES):
    import ml_dtypes
    bf16 = ml_dtypes.bfloat16

    nloc = n // ncores
    nw = (nloc + P - 1) // P
    npad = nw * P

    src = np.asarray(edge_index[0], dtype=np.int64)
    dst = np.asarray(edge_index[1], dtype=np.int64)
    x = np.asarray(x, dtype=np.float32)
    el = np.asarray(edge_len, dtype=np.float32)[:, 0]

    centers = np.linspace(0.0, MAX_RADIUS, NB, dtype=np.float64)
    width = (centers[1] - centers[0]) * 0.5
    rbf_all = np.exp(-((el[:, None].astype(np.float64) - centers) ** 2)
                     / (2.0 * width ** 2)).astype(np.float32)  # [E, 16]

    core_of = dst // nloc
    per_core = []
    cnt_w = np.zeros((ncores, nw), dtype=np.int64)
    for c in range(ncores):
        eids = np.nonzero(core_of == c)[0]
        dloc = (dst[eids] - c * nloc).astype(np.int64)
        order = np.argsort(dloc, kind="stable")
        eids = eids[order]
        dloc = dloc[order]
        w_of = dloc // P
        cnt_w[c] = np.bincount(w_of, minlength=nw)
        per_core.append((eids, dloc, w_of))

    # per-window block counts, equalized across cores; total padded to x16
    # (16 blocks = one 4-group DMA chunk)
    bws = np.maximum(1, (cnt_w.max(axis=0) + P - 1) // P)  # [nw]
    bws[-1] += (-int(bws.sum())) % 16
    btot = int(bws.sum())
    epad = btot * P

    block_window = np.repeat(np.arange(nw), bws)
    boff = np.concatenate([[0], np.cumsum(bws)[:-1]])  # first block of window

    uw1f = np.asarray(uw1, np.float32)
    w2u = (np.asarray(mw2, np.float32) @ uw1f[DIN:]).astype(bf16)  # [128, 64]
    mb2u = np.asarray(mb2, np.float32) @ uw1f[DIN:]                # [64]

    in_maps = []
    for c in range(ncores):
        eids, dloc, w_of = per_core[c]
        # slot index for each edge: window base + position within window
        win_start = np.concatenate([[0], np.cumsum(cnt_w[c])[:-1]])
        pos_in_w = np.arange(len(eids)) - win_start[w_of]
        slot = boff[w_of] * P + pos_in_w  # [e_c]

        xcat = np.zeros((epad, 2 * DIN), dtype=np.float32)
        xcat[slot, :DIN] = x[src[eids]]
        xcat[slot, DIN:] = x[dst[eids]]
        xcat8 = np.ascontiguousarray(xcat.T).astype(bf16)

        rbf = np.zeros((epad, NB), dtype=np.float32)
        rbf[slot] = rbf_all[eids]
        rbf8 = np.ascontiguousarray(rbf.T).astype(bf16)

        cnt_n = np.zeros(npad, dtype=np.float32)
        cnt_n[:nloc] = np.bincount(dloc, minlength=nloc).astype(np.float32)
        inv = (1.0 / np.maximum(cnt_n, 1.0)).astype(bf16)
        has = (cnt_n > 0).astype(np.float32)

        # one-hot stream with bf16 1/cnt values: ohsT[lane, g*P + n] =
        # 1/cnt(node) iff edge slot g*P+lane scatters into window-rel node n
        inv16 = inv.view(np.uint16)
        oh16 = np.zeros((epad, P), dtype=np.uint16)
        oh16[slot, (dloc - w_of * P)] = inv16[dloc]
        ohsT = np.ascontiguousarray(
            oh16.reshape(btot, P, P).transpose(1, 0, 2).reshape(P, epad)
        ).view(bf16)

        # augmented x: rows 0:64 = x^T, row 64 = has (carries mb2u per node)
        xtaug = np.zeros((DIN + 1, npad), dtype=np.float32)
        xtaug[:DIN, :nloc] = x[c * nloc:(c + 1) * nloc].T
        xtaug[DIN] = has
        uw1x_aug = np.concatenate(
            [uw1f[:DIN], mb2u[None, :]], axis=0)  # [65, 64]

        m = {
            "xcatT": xcat8,
            "rbfT": rbf8,
            "ohsT": ohsT,
            "xtaug": xtaug.astype(bf16),
            "uw1x": uw1x_aug.astype(bf16),
            "ub1": np.asarray(ub1, np.float32).reshape(DOUT, 1).copy(),
            "mw1_sd": np.asarray(mw1, np.float32)[:2 * DIN].astype(bf16),
            "mw1_r": np.asarray(mw1, np.float32)[2 * DIN:].astype(bf16),
            "mb1": np.asarray(mb1, np.float32).reshape(2 * DOUT, 1).copy(),
            "w2u": w2u,
            "uw2": np.asarray(uw2, np.float32).astype(bf16),
            "ub2": np.asarray(ub2, np.float32).reshape(DOUT, 1).copy(),
            "lng": np.broadcast_to(np.asarray(ln_g, np.float32)[None, :],
                                   (P, DOUT)).copy(),
            "lnb": np.broadcast_to(np.asarray(ln_b, np.float32)[None, :],
                                   (P, DOUT)).copy(),
            "identf": np.eye(P, dtype=np.float32),
        }
        in_maps.append(m)

    struct = dict(n=n, nloc=nloc, nw=nw, npad=npad, btot=btot, epad=epad,
                  bws=tuple(int(v) for v in bws),
                  block_window=tuple(int(v) for v in block_window))
    return struct, in_maps


# ---------------------------------------------------------------------------
# Device program
# ---------------------------------------------------------------------------

def _build_program(struct):
    import concourse.bass as bass
    import concourse.mybir as mybir
    import concourse.tile as tile
    from concourse import bacc

    f32 = mybir.dt.float32
    bf = mybir.dt.bfloat16
    f8 = mybir.dt.float8e4
    n, nloc, nw, npad = (struct["n"], struct["nloc"], struct["nw"],
                         struct["npad"])
    btot, epad = struct["btot"], struct["epad"]
    block_window = struct["block_window"]
    ngrp = btot // 4

    # first/last block of each window
    wfirst = {}
    wlast = {}
    for g, w in enumerate(block_window):
        wfirst.setdefault(w, g)
        wlast[w] = g

    nc = bacc.Bacc("TRN2", target_bir_lowering=False, debug=False,
                   enable_asserts=False, num_devices=NCORES)

    xcatT_d = nc.dram_tensor("xcatT", [P, epad], bf, kind="ExternalInput")
    rbfT_d = nc.dram_tensor("rbfT", [NB, epad], bf, kind="ExternalInput")
    ohsT_d = nc.dram_tensor("ohsT", [P, epad], bf, kind="ExternalInput")
    xtaug_d = nc.dram_tensor("xtaug", [DIN + 1, npad], bf,
                             kind="ExternalInput")
    uw1x_d = nc.dram_tensor("uw1x", [DIN + 1, DOUT], bf,
                            kind="ExternalInput")
    ub1_d = nc.dram_tensor("ub1", [DOUT, 1], f32, kind="ExternalInput")
    mw1_sd_d = nc.dram_tensor("mw1_sd", [2 * DIN, 2 * DOUT], bf,
                              kind="ExternalInput")
    mw1_r_d = nc.dram_tensor("mw1_r", [NB, 2 * DOUT], bf,
                             kind="ExternalInput")
    mb1_d = nc.dram_tensor("mb1", [2 * DOUT, 1], f32, kind="ExternalInput")
    w2u_d = nc.dram_tensor("w2u", [2 * DOUT, DOUT], bf, kind="ExternalInput")
    uw2_d = nc.dram_tensor("uw2", [DOUT, DOUT], bf, kind="ExternalInput")
    ub2_d = nc.dram_tensor("ub2", [DOUT, 1], f32, kind="ExternalInput")
    lng_d = nc.dram_tensor("lng", [P, DOUT], f32, kind="ExternalInput")
    lnb_d = nc.dram_tensor("lnb", [P, DOUT], f32, kind="ExternalInput")
    identf_d = nc.dram_tensor("identf", [P, P], f32, kind="ExternalInput")
    out_d = nc.dram_tensor("out", [npad, DOUT], f32, kind="ExternalOutput")

    AX = mybir.AxisListType
    OP = mybir.AluOpType
    ACT = mybir.ActivationFunctionType

    with tile.TileContext(nc) as tc:
        with (
            tc.tile_pool(name="const", bufs=1) as cpool,
            tc.tile_pool(name="gath", bufs=5) as gpool,
            tc.tile_pool(name="work", bufs=7) as wpool,
            tc.tile_pool(name="ph", bufs=3, space="PSUM") as ph_pool,
            tc.tile_pool(name="pm", bufs=3, space="PSUM") as pm_pool,
            tc.tile_pool(name="pa", bufs=2, space="PSUM") as pa_pool,
        ):
            def cload(dram, shape, dtype=f32):
                t = cpool.tile(shape, dtype, name=dram.name + "_t")
                nc.sync.dma_start(out=t[:], in_=dram[:])
                return t

            # critical consts first (needed by the first edge groups)
            mw1_sd_t = cload(mw1_sd_d, [2 * DIN, 2 * DOUT], bf)
            mw1_r_t = cload(mw1_r_d, [NB, 2 * DOUT], bf)
            mb1_t = cload(mb1_d, [2 * DOUT, 1])
            w2u_t = cload(w2u_d, [2 * DOUT, DOUT], bf)
            uw1x_t = cload(uw1x_d, [DIN + 1, DOUT], bf)
            ub1_t = cload(ub1_d, [DOUT, 1])

            # pipeline state
            st = {}      # k -> dict(ph, hT, pm, msg)
            chunks = {}  # c -> (xc4, rb4, oh4)
            pa_cur = {}
            CH = 4 * 4 * P  # edges per DMA chunk (4 groups)
            nch = ngrp // 4

            def stage_dma(c):
                e0 = c * CH
                xc4 = gpool.tile([P, CH], bf, tag="xc", name=f"xc_{c}")
                nc.sync.dma_start(out=xc4[:], in_=xcatT_d[:, e0:e0 + CH])
                oh4 = gpool.tile([P, CH], bf, tag="oh", name=f"oh_{c}")
                nc.sync.dma_start(out=oh4[:], in_=ohsT_d[:, e0:e0 + CH])
                rb4 = gpool.tile([NB, CH], bf, tag="rb", name=f"rb_{c}")
                nc.sync.dma_start(out=rb4[:], in_=rbfT_d[:, e0:e0 + CH])
                chunks[c] = (xc4, rb4, oh4)

            stage_dma(0)
            stage_dma(1)

            # secondary consts (flushes / end phase)
            xtaug_t = cload(xtaug_d, [DIN + 1, npad], bf)
            uw2_t = cload(uw2_d, [DOUT, DOUT], bf)
            ub2_t = cload(ub2_d, [DOUT, 1])
            lng_t = cload(lng_d, [P, DOUT])
            lnb_t = cload(lnb_d, [P, DOUT])
            identf_t = cload(identf_d, [P, P])
            eps_t = cpool.tile([P, 1], f32, name="eps_t")
            nc.vector.memset(eps_t[:], 1e-5)

            # u1 hidden activations for all local nodes, filled per window
            u1_sb = cpool.tile([DOUT, npad], bf, name="u1_sb")

            def stage_l1mm(k):
                s = st[k] = {}
                xc4, rb4, _ = chunks[k // 4]
                q = (k % 4) * 4 * P
                ph = s["ph"] = ph_pool.tile([P, 4 * P], f32, tag="ph",
                                            name=f"ph_{k}")
                nc.tensor.matmul(ph[:], mw1_sd_t[:], xc4[:, q:q + 4 * P],
                                 start=True, stop=False)
                nc.tensor.matmul(ph[:], mw1_r_t[:], rb4[:, q:q + 4 * P],
                                 start=False, stop=True)

            def stage_silu(k):
                s = st[k]
                hT = s["hT"] = wpool.tile([P, 4 * P], bf, tag="hT",
                                          name=f"hT_{k}")
                nc.scalar.activation(out=hT[:], in_=s["ph"][:],
                                     func=ACT.Silu, bias=mb1_t[:, 0:1])

            def stage_l2mm(k):
                s = st[k]
                hT = s["hT"]
                pm = s["pm"] = pm_pool.tile([P, 4 * DOUT], f32, tag="pm",
                                            name=f"pm_{k}")
                for j in range(4):
                    nc.tensor.matmul(pm[:, j * DOUT:(j + 1) * DOUT],
                                     hT[:, j * P:(j + 1) * P],
                                     w2u_t[:], start=True, stop=True)

            def stage_copy(k):
                s = st[k]
                msg = s["msg"] = wpool.tile([P, 4 * DOUT], bf, tag="msg",
                                            name=f"msg_{k}")
                nc.vector.tensor_copy(out=msg[:], in_=s["pm"][:])

            def stage_scatter(k):
                s = st[k]
                msg = s["msg"]
                oh4 = chunks[k // 4][2]
                q = (k % 4) * 4 * P
                for j in range(4):
                    g = 4 * k + j
                    w = block_window[g]
                    if g == wfirst[w]:
                        pa_cur[w] = pa_pool.tile([DOUT, P], f32, tag="pa",
                                                 name=f"pa_w{w}")
                    nc.tensor.matmul(
                        pa_cur[w][:],
                        msg[:, j * DOUT:(j + 1) * DOUT],
                        oh4[:, q + j * P:q + (j + 1) * P],
                        start=(g == wfirst[w]), stop=False,
                        skip_group_check=True)
                    if g != wlast[w]:
                        continue
                    wc = slice(w * P, (w + 1) * P)
                    # add uw1_x^T @ [x; has] and close the PSUM group
                    nc.tensor.matmul(pa_cur[w][:], uw1x_t[:],
                                     xtaug_t[:, wc], start=False, stop=True,
                                     skip_group_check=True)
                    nc.scalar.activation(out=u1_sb[:, wc], in_=pa_cur[w][:],
                                         func=ACT.Silu, bias=ub1_t[:, 0:1])
                    del pa_cur[w]
                del st[k]

            for k in range(ngrp + 7):
                if 3 <= k < ngrp + 3:
                    stage_l2mm(k - 3)
                if 1 <= k < ngrp + 1:
                    stage_silu(k - 1)
                if k >= 7:
                    stage_scatter(k - 7)
                if 5 <= k < ngrp + 5:
                    stage_copy(k - 5)
                if k < ngrp:
                    if k % 4 == 0 and k // 4 + 2 < nch:
                        stage_dma(k // 4 + 2)
                    stage_l1mm(k)

            # ---------- update layer 2 + LayerNorm (pipelined end phase) ---
            ust = {}
            nug = (npad + 511) // 512

            def stage_upB(m):
                u0 = m * 512
                cw = min(512, npad - u0)
                s = ust[m] = {"cw": cw, "u0": u0}
                pz = s["pz"] = ph_pool.tile([P, 512], f32, tag="ph",
                                            name=f"pz_{m}")
                nc.tensor.matmul(pz[0:DOUT, 0:cw], uw2_t[:],
                                 u1_sb[:, u0:u0 + cw], start=True, stop=True)
                zT = s["zT"] = wpool.tile([DOUT, 512], f32, tag="zT",
                                          name=f"zT_{m}")
                nc.scalar.activation(out=zT[:, 0:cw], in_=pz[0:DOUT, 0:cw],
                                     func=ACT.Identity, bias=ub2_t[:, 0:1])

            def stage_upC(m):
                s = ust[m]
                cw, u0 = s["cw"], s["u0"]
                nj = cw // P
                zT = s["zT"]
                pz2 = pm_pool.tile([P, 4 * DOUT], f32, tag="pm",
                                   name=f"pz2_{m}")
                for j in range(nj):
                    nc.tensor.transpose(
                        out=pz2[:, j * DOUT:(j + 1) * DOUT],
                        in_=zT[:, j * P:(j + 1) * P],
                        identity=identf_t[0:DOUT, 0:DOUT])
                # LayerNorm on [128, nj, 64] (free-axis per-node)
                zc = wpool.tile([P, 4 * DOUT], f32, tag="zc", name=f"zc_{m}")
                red = wpool.tile([P, 4], f32, tag="red", name=f"red_{m}")
                red2 = wpool.tile([P, 4], f32, tag="red2", name=f"red2_{m}")
                z3 = pz2[:, 0:nj * DOUT].rearrange("p (j d) -> p j d", d=DOUT)
                nc.vector.tensor_reduce(out=red[:, 0:nj], in_=z3, axis=AX.X,
                                        op=OP.add)
                nc.vector.tensor_scalar_mul(red[:, 0:nj], red[:, 0:nj],
                                            -1.0 / DOUT)
                zc3 = zc[:, 0:nj * DOUT].rearrange("p (j d) -> p j d", d=DOUT)
                nc.vector.tensor_tensor(
                    out=zc3, in0=z3,
                    in1=red[:, 0:nj, None].to_broadcast([P, nj, DOUT]),
                    op=OP.add)
                sq = wpool.tile([P, 4 * DOUT], f32, tag="sq", name=f"sq_{m}")
                sq3 = sq[:, 0:nj * DOUT].rearrange("p (j d) -> p j d", d=DOUT)
                nc.vector.tensor_tensor(out=sq3, in0=zc3, in1=zc3, op=OP.mult)
                nc.vector.tensor_reduce(out=red2[:, 0:nj], in_=sq3, axis=AX.X,
                                        op=OP.add)
                sd = wpool.tile([P, 4], f32, tag="sd", name=f"sd_{m}")
                nc.scalar.activation(out=sd[:, 0:nj], in_=red2[:, 0:nj],
                                     func=ACT.Sqrt, scale=1.0 / DOUT,
                                     bias=eps_t[:, 0:1])
                rs = wpool.tile([P, 4], f32, tag="rs", name=f"rs_{m}")
                nc.vector.reciprocal(out=rs[:, 0:nj], in_=sd[:, 0:nj])
                zn = wpool.tile([P, 4 * DOUT], f32, tag="zn", name=f"zn_{m}")
                zn3 = zn[:, 0:nj * DOUT].rearrange("p (j d) -> p j d", d=DOUT)
                nc.vector.tensor_tensor(
                    out=zn3, in0=zc3,
                    in1=rs[:, 0:nj, None].to_broadcast([P, nj, DOUT]),
                    op=OP.mult)
                for j in range(nj):
                    js = slice(j * DOUT, (j + 1) * DOUT)
                    nc.vector.tensor_tensor(out=zn[:, js], in0=zn[:, js],
                                            in1=lng_t[:], op=OP.mult)
                    nc.vector.tensor_tensor(out=zn[:, js], in0=zn[:, js],
                                            in1=lnb_t[:], op=OP.add)
                nc.sync.dma_start(
                    out=out_d[u0:u0 + cw].rearrange("(j p) d -> p j d", p=P),
                    in_=zn[:, 0:nj * DOUT].rearrange("p (j d) -> p j d",
                                                     d=DOUT))
                del ust[m]

            for m in range(nug + 2):
                if m < nug:
                    stage_upB(m)
                if m >= 2:
                    stage_upC(m - 2)

    nc.compile()
    return nc


# ---------------------------------------------------------------------------
# Entry point
# ---------------------------------------------------------------------------

last_results = None


def kernel(x, edge_index, edge_vec, edge_len,
           mw1, mb1, mw2, mb2, uw1, ub1, uw2, ub2, ln_g, ln_b):
    global last_results
    import os
    from concourse.bass_utils import run_bass_kernel_spmd

    struct, in_maps = _build_host_data(
        x, edge_index, edge_len, mw1, mb1, mw2, mb2,
        uw1, ub1, uw2, ub2, ln_g, ln_b)

    key = (struct["n"], struct["btot"], struct["bws"])
    if key not in _prog_cache:
        _prog_cache[key] = _build_program(struct)
    nc = _prog_cache[key]

    kw = {}
    if os.environ.get("K_TRACE", ""):
        import profile_shim
        profile_shim.install()
        kw = dict(trace=True, trace_cores=list(range(NCORES)),
                  tmpdir="/tmp/ntff_out")
    res = run_bass_kernel_spmd(nc, in_maps, core_ids=list(range(NCORES)), **kw)
    last_results = res
    nloc = struct["nloc"]
    out = np.concatenate([res.results[c]["out"][:nloc] for c in range(NCORES)],
                         axis=0)
    return out.astype(np.float32)


# revision 80
# speedup vs baseline: 1.4982x; 1.0117x over previous
"""GNN message-passing layer (EquivariantMPLayer) on 8 Trainium2 NeuronCores.

Sharding: edges are sharded by destination-node range (dst // (N/8)) so each
core aggregates its own node range locally -- no collectives needed. Per core,
edges are sorted by dst and grouped into 128-node windows; each window's edge
list is padded to 128-edge blocks. Per-window block counts are equalized
across cores (max over cores) so a single SPMD program serves all 8 cores.

The host pre-gathers x[src] and x[dst] for every edge slot into a single
feature-major stream xcatT [128, epad] (rows 0:64 = src feats, 64:128 = dst
feats, columns in device consumption order), and pre-builds the per-block
scatter one-hots as a second stream ohsT [128, epad] whose nonzero VALUES are
1/cnt(dst) -- so the scatter matmul directly produces the mean aggregate.

Algebraic folds: the L2 weight is W2U = mw2 @ uw1_agg, so the scatter PSUM
accumulates uw1_agg^T @ (agg/cnt) -- the update-MLP's aggregation term --
directly. When a window closes, one extra matmul adds uw1_x^T @ x (with an
augmented constant row supplying has*uw1_agg^T@mb2 exactly), and a single
ACT Silu produces the u1 hidden vector for those 128 nodes. The second
update layer + LayerNorm run as a short pipelined end phase.

The device pipeline is software-pipelined so the tensor engine never waits:
at iteration k it runs L1(k), L2(k-2) and scatter(k-6), while ACT runs
silu(k) and the per-window u1 activation, and DVE only copies msg out of
PSUM.
"""

import numpy as np

N = 50000
E = 800000
DIN = 64
DOUT = 64
NB = 16
MAX_RADIUS = 10.0
NCORES = 8
P = 128

_prog_cache = {}


# ---------------------------------------------------------------------------
# Host-side structure / metadata
# ---------------------------------------------------------------------------

def _build_host_data(x, edge_index, edge_len, mw1, mb1, mw2, mb2,
                     uw1, ub1, uw2, ub2, ln_g, ln_b,
                     n=N, ncores=NCORES):
    import ml_dtypes
    bf16 = ml_dtypes.bfloat16

    nloc = n // ncores
    nw = (nloc + P - 1) // P
    npad = nw * P

    src = np.asarray(edge_index[0], dtype=np.int64)
    dst = np.asarray(edge_index[1], dtype=np.int64)
    x = np.asarray(x, dtype=np.float32)
    el = np.asarray(edge_len, dtype=np.float32)[:, 0]

    centers = np.linspace(0.0, MAX_RADIUS, NB, dtype=np.float64)
    width = (centers[1] - centers[0]) * 0.5
    rbf_all = np.exp(-((el[:, None].astype(np.float64) - centers) ** 2)
                     / (2.0 * width ** 2)).astype(np.float32)  # [E, 16]

    core_of = dst // nloc
    per_core = []
    cnt_w = np.zeros((ncores, nw), dtype=np.int64)
    for c in range(ncores):
        eids = np.nonzero(core_of == c)[0]
        dloc = (dst[eids] - c * nloc).astype(np.int64)
        order = np.argsort(dloc, kind="stable")
        eids = eids[order]
        dloc = dloc[order]
        w_of = dloc // P
        cnt_w[c] = np.bincount(w_of, minlength=nw)
        per_core.append((eids, dloc, w_of))

    # per-window block counts, equalized across cores; total padded to x16
    # (16 blocks = one 4-group DMA chunk)
    bws = np.maximum(1, (cnt_w.max(axis=0) + P - 1) // P)  # [nw]
    bws[-1] += (-int(bws.sum())) % 16
    btot = int(bws.sum())
    epad = btot * P

    block_window = np.repeat(np.arange(nw), bws)
    boff = np.concatenate([[0], np.cumsum(bws)[:-1]])  # first block of window

    uw1f = np.asarray(uw1, np.float32)
    w2u = (np.asarray(mw2, np.float32) @ uw1f[DIN:]).astype(bf16)  # [128, 64]
    mb2u = np.asarray(mb2, np.float32) @ uw1f[DIN:]                # [64]

    in_maps = []
    for c in range(ncores):
        eids, dloc, w_of = per_core[c]
        # slot index for each edge: window base + position within window
        win_start = np.concatenate([[0], np.cumsum(cnt_w[c])[:-1]])
        pos_in_w = np.arange(len(eids)) - win_start[w_of]
        slot = boff[w_of] * P + pos_in_w  # [e_c]

        xcat = np.zeros((epad, 2 * DIN), dtype=np.float32)
        xcat[slot, :DIN] = x[src[eids]]
        xcat[slot, DIN:] = x[dst[eids]]
        xcat8 = np.ascontiguousarray(xcat.T).astype(bf16)

        rbf = np.zeros((epad, NB), dtype=np.float32)
        rbf[slot] = rbf_all[eids]
        rbf8 = np.ascontiguousarray(rbf.T).astype(bf16)

        import ml_dtypes as _md
        f8 = _md.float8_e4m3
        cnt_n = np.zeros(npad, dtype=np.float32)
        cnt_n[:nloc] = np.bincount(dloc, minlength=nloc).astype(np.float32)
        inv8 = (1.0 / np.maximum(cnt_n, 1.0)).astype(f8).view(np.uint8)
        has = (cnt_n > 0).astype(np.float32)

        # one-hot stream with fp8 1/cnt values: ohsT[lane, g*P + n] =
        # 1/cnt(node) iff edge slot g*P+lane scatters into window-rel node n
        oh8 = np.zeros((epad, P), dtype=np.uint8)
        oh8[slot, (dloc - w_of * P)] = inv8[dloc]
        ohsT = np.ascontiguousarray(
            oh8.reshape(btot, P, P).transpose(1, 0, 2).reshape(P, epad)
        ).view(f8)

        # augmented x: rows 0:64 = x^T, row 64 = has (carries mb2u per node)
        xtaug = np.zeros((DIN + 1, npad), dtype=np.float32)
        xtaug[:DIN, :nloc] = x[c * nloc:(c + 1) * nloc].T
        xtaug[DIN] = has
        uw1x_aug = np.concatenate(
            [uw1f[:DIN], mb2u[None, :]], axis=0)  # [65, 64]

        m = {
            "xcatT": xcat8,
            "rbfT": rbf8,
            "ohsT": ohsT,
            "xtaug": xtaug.astype(bf16),
            "uw1x": uw1x_aug.astype(bf16),
            "ub1": np.asarray(ub1, np.float32).reshape(DOUT, 1).copy(),
            "mw1_sd": np.asarray(mw1, np.float32)[:2 * DIN].astype(bf16),
            "mw1_r": np.asarray(mw1, np.float32)[2 * DIN:].astype(bf16),
            "mb1": np.asarray(mb1, np.float32).reshape(2 * DOUT, 1).copy(),
            "w2u": w2u,
            "uw2": np.asarray(uw2, np.float32).astype(bf16),
            "ub2": np.asarray(ub2, np.float32).reshape(DOUT, 1).copy(),
            "lng": np.broadcast_to(np.asarray(ln_g, np.float32)[None, :],
                                   (P, DOUT)).copy(),
            "lnb": np.broadcast_to(np.asarray(ln_b, np.float32)[None, :],
                                   (P, DOUT)).copy(),
            "identf": np.eye(P, dtype=np.float32),
        }
        in_maps.append(m)

    struct = dict(n=n, nloc=nloc, nw=nw, npad=npad, btot=btot, epad=epad,
                  bws=tuple(int(v) for v in bws),
                  block_window=tuple(int(v) for v in block_window))
    return struct, in_maps


# ---------------------------------------------------------------------------
# Device program
# ---------------------------------------------------------------------------

def _build_program(struct):
    import concourse.bass as bass
    import concourse.mybir as mybir
    import concourse.tile as tile
    from concourse import bacc

    f32 = mybir.dt.float32
    bf = mybir.dt.bfloat16
    f8 = mybir.dt.float8e4
    n, nloc, nw, npad = (struct["n"], struct["nloc"], struct["nw"],
                         struct["npad"])
    btot, epad = struct["btot"], struct["epad"]
    block_window = struct["block_window"]
    ngrp = btot // 4

    # first/last block of each window
    wfirst = {}
    wlast = {}
    for g, w in enumerate(block_window):
        wfirst.setdefault(w, g)
        wlast[w] = g

    nc = bacc.Bacc("TRN2", target_bir_lowering=False, debug=False,
                   enable_asserts=False, num_devices=NCORES)

    xcatT_d = nc.dram_tensor("xcatT", [P, epad], bf, kind="ExternalInput")
    rbfT_d = nc.dram_tensor("rbfT", [NB, epad], bf, kind="ExternalInput")
    ohsT_d = nc.dram_tensor("ohsT", [P, epad], f8, kind="ExternalInput")
    xtaug_d = nc.dram_tensor("xtaug", [DIN + 1, npad], bf,
                             kind="ExternalInput")
    uw1x_d = nc.dram_tensor("uw1x", [DIN + 1, DOUT], bf,
                            kind="ExternalInput")
    ub1_d = nc.dram_tensor("ub1", [DOUT, 1], f32, kind="ExternalInput")
    mw1_sd_d = nc.dram_tensor("mw1_sd", [2 * DIN, 2 * DOUT], bf,
                              kind="ExternalInput")
    mw1_r_d = nc.dram_tensor("mw1_r", [NB, 2 * DOUT], bf,
                             kind="ExternalInput")
    mb1_d = nc.dram_tensor("mb1", [2 * DOUT, 1], f32, kind="ExternalInput")
    w2u_d = nc.dram_tensor("w2u", [2 * DOUT, DOUT], bf, kind="ExternalInput")
    uw2_d = nc.dram_tensor("uw2", [DOUT, DOUT], bf, kind="ExternalInput")
    ub2_d = nc.dram_tensor("ub2", [DOUT, 1], f32, kind="ExternalInput")
    lng_d = nc.dram_tensor("lng", [P, DOUT], f32, kind="ExternalInput")
    lnb_d = nc.dram_tensor("lnb", [P, DOUT], f32, kind="ExternalInput")
    identf_d = nc.dram_tensor("identf", [P, P], f32, kind="ExternalInput")
    out_d = nc.dram_tensor("out", [npad, DOUT], f32, kind="ExternalOutput")

    AX = mybir.AxisListType
    OP = mybir.AluOpType
    ACT = mybir.ActivationFunctionType

    with tile.TileContext(nc) as tc:
        with (
            tc.tile_pool(name="const", bufs=1) as cpool,
            tc.tile_pool(name="gath", bufs=5) as gpool,
            tc.tile_pool(name="work", bufs=7) as wpool,
            tc.tile_pool(name="ph", bufs=3, space="PSUM") as ph_pool,
            tc.tile_pool(name="pm", bufs=3, space="PSUM") as pm_pool,
            tc.tile_pool(name="pa", bufs=2, space="PSUM") as pa_pool,
        ):
            def cload(dram, shape, dtype=f32):
                t = cpool.tile(shape, dtype, name=dram.name + "_t")
                nc.sync.dma_start(out=t[:], in_=dram[:])
                return t

            # critical consts first (needed by the first edge groups)
            mw1_sd_t = cload(mw1_sd_d, [2 * DIN, 2 * DOUT], bf)
            mw1_r_t = cload(mw1_r_d, [NB, 2 * DOUT], bf)
            mb1_t = cload(mb1_d, [2 * DOUT, 1])
            w2u_t = cload(w2u_d, [2 * DOUT, DOUT], bf)
            uw1x_t = cload(uw1x_d, [DIN + 1, DOUT], bf)
            ub1_t = cload(ub1_d, [DOUT, 1])

            # pipeline state
            st = {}      # k -> dict(ph, hT, pm, msg)
            chunks = {}  # c -> (xc4, rb4, oh4)
            pa_cur = {}
            CH = 4 * 4 * P  # edges per DMA chunk (4 groups)
            nch = ngrp // 4

            def stage_dma(c):
                e0 = c * CH
                xc4 = gpool.tile([P, CH], bf, tag="xc", name=f"xc_{c}")
                nc.sync.dma_start(out=xc4[:], in_=xcatT_d[:, e0:e0 + CH])
                oh4 = gpool.tile([P, CH], f8, tag="oh", name=f"oh_{c}")
                nc.sync.dma_start(out=oh4[:], in_=ohsT_d[:, e0:e0 + CH])
                rb4 = gpool.tile([NB, CH], bf, tag="rb", name=f"rb_{c}")
                nc.sync.dma_start(out=rb4[:], in_=rbfT_d[:, e0:e0 + CH])
                chunks[c] = (xc4, rb4, oh4)

            stage_dma(0)
            stage_dma(1)

            # secondary consts (flushes / end phase)
            xtaug_t = cload(xtaug_d, [DIN + 1, npad], bf)
            uw2_t = cload(uw2_d, [DOUT, DOUT], bf)
            ub2_t = cload(ub2_d, [DOUT, 1])
            lng_t = cload(lng_d, [P, DOUT])
            lnb_t = cload(lnb_d, [P, DOUT])
            identf_t = cload(identf_d, [P, P])
            eps_t = cpool.tile([P, 1], f32, name="eps_t")
            nc.vector.memset(eps_t[:], 1e-5)

            # u1 hidden activations for all local nodes, filled per window
            u1_sb = cpool.tile([DOUT, npad], bf, name="u1_sb")

            def stage_l1mm(k):
                s = st[k] = {}
                xc4, rb4, _ = chunks[k // 4]
                q = (k % 4) * 4 * P
                ph = s["ph"] = ph_pool.tile([P, 4 * P], f32, tag="ph",
                                            name=f"ph_{k}")
                nc.tensor.matmul(ph[:], mw1_sd_t[:], xc4[:, q:q + 4 * P],
                                 start=True, stop=False)
                nc.tensor.matmul(ph[:], mw1_r_t[:], rb4[:, q:q + 4 * P],
                                 start=False, stop=True)

            def stage_silu(k):
                s = st[k]
                hT = s["hT"] = wpool.tile([P, 4 * P], bf, tag="hT",
                                          name=f"hT_{k}")
                nc.scalar.activation(out=hT[:], in_=s["ph"][:],
                                     func=ACT.Silu, bias=mb1_t[:, 0:1])

            def stage_l2mm(k):
                s = st[k]
                hT = s["hT"]
                pm = s["pm"] = pm_pool.tile([P, 4 * DOUT], f32, tag="pm",
                                            name=f"pm_{k}")
                for j in range(4):
                    nc.tensor.matmul(pm[:, j * DOUT:(j + 1) * DOUT],
                                     hT[:, j * P:(j + 1) * P],
                                     w2u_t[:], start=True, stop=True)

            def stage_copy(k):
                s = st[k]
                msg = s["msg"] = wpool.tile([P, 4 * DOUT], bf, tag="msg",
                                            name=f"msg_{k}")
                nc.vector.tensor_copy(out=msg[:], in_=s["pm"][:])

            def stage_scatter(k):
                s = st[k]
                msg = s["msg"]
                oh4 = chunks[k // 4][2]
                q = (k % 4) * 4 * P
                for j in range(4):
                    g = 4 * k + j
                    w = block_window[g]
                    if g == wfirst[w]:
                        pa_cur[w] = pa_pool.tile([DOUT, P], f32, tag="pa",
                                                 name=f"pa_w{w}")
                    nc.tensor.matmul(
                        pa_cur[w][:],
                        msg[:, j * DOUT:(j + 1) * DOUT],
                        oh4[:, q + j * P:q + (j + 1) * P],
                        start=(g == wfirst[w]), stop=False,
                        skip_group_check=True)
                    if g != wlast[w]:
                        continue
                    wc = slice(w * P, (w + 1) * P)
                    # add uw1_x^T @ [x; has] and close the PSUM group
                    nc.tensor.matmul(pa_cur[w][:], uw1x_t[:],
                                     xtaug_t[:, wc], start=False, stop=True,
                                     skip_group_check=True)
                    nc.scalar.activation(out=u1_sb[:, wc], in_=pa_cur[w][:],
                                         func=ACT.Silu, bias=ub1_t[:, 0:1])
                    del pa_cur[w]
                del st[k]

            for k in range(ngrp + 7):
                if 3 <= k < ngrp + 3:
                    stage_l2mm(k - 3)
                if 1 <= k < ngrp + 1:
                    stage_silu(k - 1)
                if k >= 7:
                    stage_scatter(k - 7)
                if 5 <= k < ngrp + 5:
                    stage_copy(k - 5)
                if k < ngrp:
                    if k % 4 == 0 and k // 4 + 2 < nch:
                        stage_dma(k // 4 + 2)
                    stage_l1mm(k)

            # ---------- update layer 2 + LayerNorm (pipelined end phase) ---
            ust = {}
            nug = (npad + 511) // 512

            def stage_upB(m):
                u0 = m * 512
                cw = min(512, npad - u0)
                s = ust[m] = {"cw": cw, "u0": u0}
                pz = s["pz"] = ph_pool.tile([P, 512], f32, tag="ph",
                                            name=f"pz_{m}")
                nc.tensor.matmul(pz[0:DOUT, 0:cw], uw2_t[:],
                                 u1_sb[:, u0:u0 + cw], start=True, stop=True)
                zT = s["zT"] = wpool.tile([DOUT, 512], f32, tag="zT",
                                          name=f"zT_{m}")
                nc.scalar.activation(out=zT[:, 0:cw], in_=pz[0:DOUT, 0:cw],
                                     func=ACT.Identity, bias=ub2_t[:, 0:1])

            def stage_upC(m):
                s = ust[m]
                cw, u0 = s["cw"], s["u0"]
                nj = cw // P
                zT = s["zT"]
                pz2 = pm_pool.tile([P, 4 * DOUT], f32, tag="pm",
                                   name=f"pz2_{m}")
                for j in range(nj):
                    nc.tensor.transpose(
                        out=pz2[:, j * DOUT:(j + 1) * DOUT],
                        in_=zT[:, j * P:(j + 1) * P],
                        identity=identf_t[0:DOUT, 0:DOUT])
                # LayerNorm on [128, nj, 64] (free-axis per-node)
                zc = wpool.tile([P, 4 * DOUT], f32, tag="zc", name=f"zc_{m}")
                red = wpool.tile([P, 4], f32, tag="red", name=f"red_{m}")
                red2 = wpool.tile([P, 4], f32, tag="red2", name=f"red2_{m}")
                z3 = pz2[:, 0:nj * DOUT].rearrange("p (j d) -> p j d", d=DOUT)
                nc.vector.tensor_reduce(out=red[:, 0:nj], in_=z3, axis=AX.X,
                                        op=OP.add)
                nc.vector.tensor_scalar_mul(red[:, 0:nj], red[:, 0:nj],
                                            -1.0 / DOUT)
                zc3 = zc[:, 0:nj * DOUT].rearrange("p (j d) -> p j d", d=DOUT)
                nc.vector.tensor_tensor(
                    out=zc3, in0=z3,
                    in1=red[:, 0:nj, None].to_broadcast([P, nj, DOUT]),
                    op=OP.add)
                sq = wpool.tile([P, 4 * DOUT], f32, tag="sq", name=f"sq_{m}")
                sq3 = sq[:, 0:nj * DOUT].rearrange("p (j d) -> p j d", d=DOUT)
                nc.vector.tensor_tensor(out=sq3, in0=zc3, in1=zc3, op=OP.mult)
                nc.vector.tensor_reduce(out=red2[:, 0:nj], in_=sq3, axis=AX.X,
                                        op=OP.add)
                sd = wpool.tile([P, 4], f32, tag="sd", name=f"sd_{m}")
                nc.scalar.activation(out=sd[:, 0:nj], in_=red2[:, 0:nj],
                                     func=ACT.Sqrt, scale=1.0 / DOUT,
                                     bias=eps_t[:, 0:1])
                rs = wpool.tile([P, 4], f32, tag="rs", name=f"rs_{m}")
                nc.vector.reciprocal(out=rs[:, 0:nj], in_=sd[:, 0:nj])
                zn = wpool.tile([P, 4 * DOUT], f32, tag="zn", name=f"zn_{m}")
                zn3 = zn[:, 0:nj * DOUT].rearrange("p (j d) -> p j d", d=DOUT)
                nc.vector.tensor_tensor(
                    out=zn3, in0=zc3,
                    in1=rs[:, 0:nj, None].to_broadcast([P, nj, DOUT]),
                    op=OP.mult)
                for j in range(nj):
                    js = slice(j * DOUT, (j + 1) * DOUT)
                    nc.vector.tensor_tensor(out=zn[:, js], in0=zn[:, js],
                                            in1=lng_t[:], op=OP.mult)
                    nc.vector.tensor_tensor(out=zn[:, js], in0=zn[:, js],
                                            in1=lnb_t[:], op=OP.add)
                nc.sync.dma_start(
                    out=out_d[u0:u0 + cw].rearrange("(j p) d -> p j d", p=P),
                    in_=zn[:, 0:nj * DOUT].rearrange("p (j d) -> p j d",
                                                     d=DOUT))
                del ust[m]

            for m in range(nug + 2):
                if m < nug:
                    stage_upB(m)
                if m >= 2:
                    stage_upC(m - 2)

    nc.compile()
    return nc


# ---------------------------------------------------------------------------
# Entry point
# ---------------------------------------------------------------------------

last_results = None


def kernel(x, edge_index, edge_vec, edge_len,
           mw1, mb1, mw2, mb2, uw1, ub1, uw2, ub2, ln_g, ln_b):
    global last_results
    import os
    from concourse.bass_utils import run_bass_kernel_spmd

    struct, in_maps = _build_host_data(
        x, edge_index, edge_len, mw1, mb1, mw2, mb2,
        uw1, ub1, uw2, ub2, ln_g, ln_b)

    key = (struct["n"], struct["btot"], struct["bws"])
    if key not in _prog_cache:
        _prog_cache[key] = _build_program(struct)
    nc = _prog_cache[key]

    kw = {}
    if os.environ.get("K_TRACE", ""):
        import profile_shim
        profile_shim.install()
        kw = dict(trace=True, trace_cores=list(range(NCORES)),
                  tmpdir="/tmp/ntff_out")
    res = run_bass_kernel_spmd(nc, in_maps, core_ids=list(range(NCORES)), **kw)
    last_results = res
    nloc = struct["nloc"]
    out = np.concatenate([res.results[c]["out"][:nloc] for c in range(NCORES)],
                         axis=0)
    return out.astype(np.float32)
